# revision 26
# baseline (speedup 1.0000x reference)
"""Galerkin attention (ragged graph segments) on 8 Trainium2 NeuronCores.

Math (per reference):
  qkv = x @ w_qkv.T ; split q,k,v -> [B, H, N, DH]
  k, v  <- LayerNorm over DH (eps=1e-6, affine)
  per graph g (sorted contiguous segments of N): ktv[g] = k_g^T v_g
  out_n = (q_n / size(g(n))) @ ktv[g(n)]
  y = out @ w_out.T + b_out

Sharding: 32 graphs are bin-packed onto 8 cores x S slots; every core runs
the identical instruction stream (SPMD) over T = sum(L_s) 128-row tiles per
batch entry, where L_s is the max tile count of slot s across cores. Ragged
graph ends are zero-padded; padding is neutralized by folding a 0/1 mask
into the LN scale (a = mask/std) and the final per-node output scale.

v2 dataflow per 128-node tile (engines balanced, PE kept streaming):
  PE : qkv projection (f32r, 512-free), full-cross pair k^T v (fp32,
       128-free, single PSUM bank per slot), per-slot Mf = ktv @ w_out.T,
       phase-2 out = q^T.T @ Mf.
  Act: squares for LN variance, sqrt(var+eps), psum->sbuf copies, out scale.
  DVE: two multi-group tensor_reduce (sum k / sum k^2 per head; reversed
       inner stride defeats the AP contiguity merge), tiny fixups, and the
       broadcast LN multiply (per-node-head scale via stride-0 AP).
  Pool(GpSimd): broadcast LN add (SBUF only; GPSIMD cannot touch PSUM).
Phase 2 of slot s-1 is emitted interleaved into phase 1 of slot s so the
in-order PE queue always has ready matmuls while the LN chain drains.
"""

import os
import sys

if "/opt/trn_rl_repo" not in sys.path:
    sys.path.insert(0, "/opt/trn_rl_repo")

import numpy as np

import concourse.bacc as bacc
import concourse.bass as bass
import concourse.mybir as mybir
import concourse.tile as tile
from concourse.bass_utils import run_bass_kernel_spmd

P = 128
B = 2
DIM = 512
HEADS = 8
DH = 64
INNER = HEADS * DH          # 512
R = 3 * INNER               # 1536
NCH = DIM // P              # 4 contraction chunks
NPAIRS = HEADS // 2         # 4 head pairs
EPS = 1e-6
N_CORES = 8
GRP = 4                     # tiles per matmul group (512-node span)
KTV_LAG = 5                 # tiles of software pipelining before k^T v
F32 = mybir.dt.float32
F32R = mybir.dt.float32r
BF16 = mybir.dt.bfloat16

_PROGRAM_CACHE: dict = {}


def _revap(src):
    """View of `src` with the innermost (contiguous) dim reversed.

    Stats are permutation-invariant; the negative stride stops the AP
    optimizer from merging the per-head groups into one flat run, which
    would turn a multi-group tensor_reduce into a single global one.
    """
    inner = list(src.ap[-1])
    assert inner[0] == 1 and inner[1] == DH
    return bass.AP(
        tensor=src.tensor,
        offset=src.offset + (DH - 1),
        ap=[list(d) for d in src.ap[:-1]] + [[-1, DH]],
    )


# ---------------------------------------------------------------------------
# host-side planning
# ---------------------------------------------------------------------------

def _plan(batch, num_graphs, n_cores):
    """Assign graphs to (core, slot) and compute the uniform slot widths."""
    batch = np.asarray(batch).astype(np.int64)
    G = int(num_graphs)
    counts = np.bincount(batch, minlength=G)[:G].astype(np.int64)
    starts = np.concatenate([[0], np.cumsum(counts)[:-1]])
    tiles_g = (counts + P - 1) // P

    # SPMD: every core executes the same T = sum_s max_c tiles, so only the
    # per-slot maxima matter. Sorting by size and filling slot s with ranks
    # [s*n_cores, (s+1)*n_cores) minimizes each slot's max simultaneously.
    S = (G + n_cores - 1) // n_cores
    order = list(np.argsort(-tiles_g, kind="stable")) + [-1] * (S * n_cores - G)
    core_graphs = [[] for _ in range(n_cores)]
    for s in range(S):
        for c in range(n_cores):
            core_graphs[c].append(int(order[s * n_cores + c]))

    Ls = []
    for s in range(S):
        L = max(
            int(tiles_g[core_graphs[c][s]]) if core_graphs[c][s] >= 0 else 0
            for c in range(n_cores)
        )
        Ls.append(max(L, 1))
    return counts, starts, core_graphs, Ls


def _pack_inputs(x, counts, starts, core_graphs, Ls, n_cores):
    T = sum(Ls)
    slot_off = np.concatenate([[0], np.cumsum(Ls)[:-1]])
    xT = np.ascontiguousarray(np.transpose(x, (0, 2, 1)))  # [B, DIM, N]
    per_core = []
    for c in range(n_cores):
        xTp = np.zeros((B, DIM, T * P), np.float32)
        qsc = np.zeros((T * P,), np.float32)
        kvm = np.zeros((T * P,), np.float32)
        for s, g in enumerate(core_graphs[c]):
            if g < 0 or counts[g] == 0:
                continue
            n0, ng = int(starts[g]), int(counts[g])
            off = int(slot_off[s]) * P
            xTp[:, :, off:off + ng] = xT[:, :, n0:n0 + ng]
            qsc[off:off + ng] = 1.0 / ng
            kvm[off:off + ng] = 1.0
        per_core.append((xTp, qsc, kvm))
    return per_core, slot_off


# ---------------------------------------------------------------------------
# device program
# ---------------------------------------------------------------------------

def _build_program(T, Ls, n_cores, ln_general, bo_zero=False):
    from contextlib import ExitStack

    nc = bacc.Bacc("TRN2", target_bir_lowering=False, debug=False,
                   num_devices=n_cores)

    xT = nc.dram_tensor("xT", [B, DIM, T * P], F32R, kind="ExternalInput")
    wq = nc.dram_tensor("wqkvT", [DIM, R], F32R, kind="ExternalInput")
    ws = nc.dram_tensor("wsum", [DIM, 2 * HEADS], F32R, kind="ExternalInput")
    wo = nc.dram_tensor("woutT", [INNER, DIM], F32R, kind="ExternalInput")
    bo = nc.dram_tensor("bout", [DIM], F32, kind="ExternalInput")
    qsc = nc.dram_tensor("qsc", [T * P], F32, kind="ExternalInput")
    kvm = nc.dram_tensor("kvm", [T * P], F32, kind="ExternalInput")
    if ln_general:
        lnp = nc.dram_tensor("lnp", [4, DH], F32, kind="ExternalInput")
    out = nc.dram_tensor("out", [B, T * P, DIM], F32, kind="ExternalOutput")

    slot_off = [0]
    for L in Ls[:-1]:
        slot_off.append(slot_off[-1] + L)

    Sqrt = mybir.ActivationFunctionType.Sqrt
    Square = mybir.ActivationFunctionType.Square
    mult = mybir.AluOpType.mult
    add = mybir.AluOpType.add
    sub = mybir.AluOpType.subtract
    X = mybir.AxisListType.X

    with ExitStack() as ctx:
        tc = ctx.enter_context(tile.TileContext(nc))
        const = ctx.enter_context(tc.tile_pool(name="const", bufs=1))

        xpool = ctx.enter_context(tc.tile_pool(name="xp", bufs=3))
        xTr = xT.ap()

        def xt_dma(b, n0, GW):
            t = xpool.tile([P, NCH, GW], F32R, name="xt", tag="xt")
            nc.sync.dma_start(
                out=t[:],
                in_=xTr[b].rearrange("(k c) n -> c k n", c=P)[:, :, n0:n0 + GW],
            )
            return t

        # first x tile group + W chunk 0 land before the bulk constants so
        # the PE pipeline starts ~20us earlier
        xt_first = xt_dma(0, 0, min(GRP, Ls[0]) * P)
        WQ = [const.tile([P, R], F32R, name=f"WQ{k}", tag=f"WQ{k}")
              for k in range(NCH)]
        wqr = wq.ap().rearrange("(k c) r -> k c r", c=P)
        for k in range(NCH):
            nc.sync.dma_start(out=WQ[k][:], in_=wqr[k])
        WS = const.tile([P, NCH, 2 * HEADS], F32R, tag="WS")
        nc.sync.dma_start(out=WS[:], in_=ws.ap().rearrange("(k c) r -> c k r", c=P))
        WO = const.tile([P, NCH, DIM], F32R, tag="WO")
        nc.sync.dma_start(out=WO[:], in_=wo.ap().rearrange("(k c) d -> c k d", c=P))
        BO = const.tile([P, DIM], F32, tag="BO")
        nc.sync.dma_start(out=BO[:], in_=bo.ap().partition_broadcast(P))
        QS = const.tile([P, T], F32, tag="QS")
        nc.sync.dma_start(out=QS[:], in_=qsc.ap().rearrange("(t p) -> p t", p=P))
        KM = const.tile([P, T], F32, tag="KM")
        nc.sync.dma_start(out=KM[:], in_=kvm.ap().rearrange("(t p) -> p t", p=P))
        EPSC = const.tile([P, 1], F32, tag="EPSC")
        nc.vector.memset(EPSC[:], EPS)
        if ln_general:
            LNP = const.tile([P, 4, DH], F32, tag="LNP")
            nc.sync.dma_start(out=LNP[:], in_=lnp.ap().partition_broadcast(P))

        sqpool = ctx.enter_context(tc.tile_pool(name="sqp", bufs=2))
        klvlp = ctx.enter_context(tc.tile_pool(name="klvlp", bufs=KTV_LAG + 2))
        stat = ctx.enter_context(tc.tile_pool(name="stat", bufs=27))
        qstash = ctx.enter_context(tc.tile_pool(name="qstash", bufs=2 * NPAIRS))
        bdsb = ctx.enter_context(tc.tile_pool(name="bd", bufs=2))
        mfsb = ctx.enter_context(tc.tile_pool(name="mf", bufs=2))
        outsb = ctx.enter_context(tc.tile_pool(name="outsb", bufs=3))

        kvps = ctx.enter_context(tc.tile_pool(name="kvps", bufs=2, space="PSUM"))
        qtps = ctx.enter_context(tc.tile_pool(name="qtps", bufs=1, space="PSUM"))
        ktps = ctx.enter_context(tc.tile_pool(name="ktps", bufs=1, space="PSUM"))
        mips = ctx.enter_context(tc.tile_pool(name="mips", bufs=1, space="PSUM"))
        mups = ctx.enter_context(tc.tile_pool(name="mups", bufs=1, space="PSUM"))

        # phase-2 emitters for the previous slot, interleaved into the next
        # slot's phase 1 to keep the PE queue stocked with ready matmuls
        pending_ph2 = []

        def emit_ph2(k=1):
            for _ in range(k):
                if pending_ph2:
                    pending_ph2.pop(0)()

        def make_ph2(b, soff, t, qts, Mf):
            ti = soff + t

            def go():
                ops = mips.tile([P, DIM], F32, tag="mi")
                for p in range(NPAIRS):
                    nc.tensor.matmul(
                        ops[:],
                        lhsT=qts[p][:, t * P:(t + 1) * P],
                        rhs=Mf[:, p, :],
                        start=(p == 0), stop=(p == NPAIRS - 1),
                    )
                ot = outsb.tile([P, DIM], F32, tag="ot")
                if bo_zero:
                    nc.scalar.mul(ot[:], ops[:], QS[:, ti:ti + 1])
                else:
                    nc.vector.scalar_tensor_tensor(
                        ot[:], ops[:], QS[:, ti:ti + 1], BO[:],
                        op0=mult, op1=add)
                nc.sync.dma_start(
                    out=out.ap()[b, ti * P:(ti + 1) * P, :], in_=ot[:])

            return go

        for b in range(B):
            for s, L in enumerate(Ls):
                soff = slot_off[s]
                ktv = ktps.tile([P, NPAIRS, P], F32, tag="ktv")
                qts = [qstash.tile([P, L * P], F32R, name=f"qts{i}", tag="qstash")
                       for i in range(NPAIRS)]
                ngroups = (L + GRP - 1) // GRP

                # deferred k^T v emitters (pipelined KTV_LAG tiles behind)
                pending_ktv = []

                def emit_ktv():
                    if pending_ktv:
                        pending_ktv.pop(0)()

                for grp in range(ngroups):
                    gt0 = grp * GRP
                    gw = min(GRP, L - gt0)
                    GW = gw * P
                    n0 = (soff + gt0) * P

                    if b == 0 and s == 0 and grp == 0:
                        xt = xt_first
                    else:
                        xt = xt_dma(b, n0, GW)

                    # q^T: stationary = W_q pair block, moving = x^T
                    for p in range(NPAIRS):
                        qtp = qtps.tile([P, GW], F32, tag="qtp")
                        for k in range(NCH):
                            nc.tensor.matmul(
                                qtp[:],
                                lhsT=WQ[k][:, p * P:(p + 1) * P],
                                rhs=xt[:, k, :],
                                start=(k == 0), stop=(k == NCH - 1),
                            )
                        nc.scalar.copy(out=qts[p][:, gt0 * P:gt0 * P + GW],
                                       in_=qtp[:])
                        emit_ph2()

                    for tl in range(gw):
                        t = gt0 + tl
                        ti = soff + t  # global tile index (mask/scale column)

                        kv = kvps.tile([P, 2, INNER], F32, tag="kv")
                        mu_ps = mups.tile([P, 2, HEADS], F32, tag="mu_ps")
                        for k in range(NCH):
                            lx = xt[:, k, tl * P:(tl + 1) * P]
                            nc.tensor.matmul(
                                kv[:, 0, :], lhsT=lx,
                                rhs=WQ[k][:, INNER:2 * INNER],
                                start=(k == 0), stop=(k == NCH - 1))
                            nc.tensor.matmul(
                                kv[:, 1, :], lhsT=lx,
                                rhs=WQ[k][:, 2 * INNER:R],
                                start=(k == 0), stop=(k == NCH - 1))
                            # per-head means, same stationary: mu = x @ Wsum/64
                            nc.tensor.matmul(
                                mu_ps[:].rearrange("p t h -> p (t h)"),
                                lhsT=lx, rhs=WS[:, k, :],
                                start=(k == 0), stop=(k == NCH - 1))

                        kv4 = kv[:].rearrange("p t (h d) -> p t h d", h=HEADS)

                        # LN variance: squares on Act, per-head sums via one
                        # multi-group reduce; mean comes from the matmul above
                        sq = sqpool.tile([P, 2, HEADS, DH + 4], F32, tag="sq")
                        nc.scalar.activation(sq[:, :, :, 0:DH], kv4, Square)
                        mus = stat.tile([P, 2, HEADS], F32, tag="mus")
                        nc.vector.tensor_copy(mus[:], mu_ps[:])
                        msq = stat.tile([P, 2, HEADS], F32, tag="msq")
                        nc.vector.tensor_reduce(out=msq[:],
                                                in_=sq[:, :, :, 0:DH],
                                                axis=X, op=add)
                        D2 = stat.tile([P, 2, HEADS], F32, tag="D2")
                        nc.vector.tensor_tensor(D2[:], mus[:], mus[:], op=mult)
                        var = stat.tile([P, 2, HEADS], F32, tag="var")
                        nc.vector.scalar_tensor_tensor(
                            var[:], msq[:], 1.0 / DH, D2[:], op0=mult, op1=sub)
                        stdt = stat.tile([P, 2, HEADS], F32, tag="stdt")
                        nc.scalar.activation(stdt[:], var[:], Sqrt,
                                             bias=EPSC[:, 0:1])
                        rstd = stat.tile([P, 2, HEADS], F32, tag="rstd")
                        nc.vector.reciprocal(rstd[:], stdt[:])
                        # c = -mu * rstd. No pad mask needed: pad rows have
                        # x = 0, so k = v = mu = 0 and klvl = 0 follows.
                        cv = stat.tile([P, 2, HEADS], BF16, tag="cv")
                        nc.vector.scalar_tensor_tensor(
                            cv[:], mus[:], -1.0, rstd[:], op0=mult, op1=mult)

                        # apply: mult on DVE (PSUM read), add on GpSimd (SBUF)
                        # klvl in bf16: feeds the k^T v matmuls at 1 cycle/row
                        klvl = klvlp.tile([P, 2, HEADS, DH], BF16, tag="klvl")
                        nc.vector.tensor_tensor(
                            klvl[:], kv4,
                            rstd[:, :, :, None].broadcast_to([P, 2, HEADS, DH]),
                            op=mult)
                        nc.gpsimd.tensor_tensor(
                            klvl[:], klvl[:],
                            cv[:, :, :, None].broadcast_to([P, 2, HEADS, DH]),
                            op=add)
                        if ln_general:
                            bmk = stat.tile([P, 2, DH], F32, tag="bmk")
                            nc.vector.tensor_scalar(bmk[:, 0], LNP[:, 1],
                                                    KM[:, ti:ti + 1], None,
                                                    op0=mult)
                            nc.vector.tensor_scalar(bmk[:, 1], LNP[:, 3],
                                                    KM[:, ti:ti + 1], None,
                                                    op0=mult)
                            for half, wi in ((0, 0), (1, 2)):
                                nc.vector.tensor_tensor(
                                    klvl[:, half], klvl[:, half],
                                    LNP[:, wi, None, :].broadcast_to(
                                        [P, HEADS, DH]), op=mult)
                                nc.vector.tensor_tensor(
                                    klvl[:, half], klvl[:, half],
                                    bmk[:, half, None, :].broadcast_to(
                                        [P, HEADS, DH]), op=add)

                        # full-cross pair k^T v (transposed: lhsT = v side),
                        # deferred KTV_LAG tiles to hide the LN chain latency
                        def make_ktv(klvl=klvl, t=t):
                            def go():
                                klf = klvl[:, 0].rearrange("p h d -> p (h d)")
                                vlf = klvl[:, 1].rearrange("p h d -> p (h d)")
                                for p in range(NPAIRS):
                                    nc.tensor.matmul(
                                        ktv[:, p, :],
                                        lhsT=vlf[:, p * P:(p + 1) * P],
                                        rhs=klf[:, p * P:(p + 1) * P],
                                        start=(t == 0 and p == 0),
                                        stop=(t == L - 1 and p == NPAIRS - 1))
                            return go

                        pending_ktv.append(make_ktv())
                        if len(pending_ktv) > KTV_LAG:
                            emit_ktv()
                        emit_ph2()

                while pending_ktv:
                    emit_ktv()
                emit_ph2(len(pending_ph2))

                # block-diag (ktv_h)^T for Mf and phase 2
                bd = bdsb.tile([P, NPAIRS, P], F32R, tag="bd")
                nc.gpsimd.memset(bd[:].bitcast(mybir.dt.uint32), 0)
                for p in range(NPAIRS):
                    nc.vector.tensor_copy(bd[0:DH, p, 0:DH],
                                          ktv[0:DH, p, 0:DH])
                    nc.vector.tensor_copy(bd[DH:P, p, DH:P],
                                          ktv[DH:P, p, DH:P])

                # Mf = blockdiag(ktv) @ w_out.T   [INNER-pair rows x DIM]
                # (psum space borrowed from the kv pool, idle at slot end)
                Mf = mfsb.tile([P, NPAIRS, DIM], F32R, tag="Mf")
                for half in range(2):
                    mfp = kvps.tile([P, 2, INNER], F32, tag="kv")
                    for i in range(2):
                        p = 2 * half + i
                        nc.tensor.matmul(mfp[:, i, :], lhsT=bd[:, p, :],
                                         rhs=WO[:, p, :], start=True, stop=True)
                    for i in range(2):
                        p = 2 * half + i
                        nc.scalar.copy(out=Mf[:, p, :], in_=mfp[:, i, :])

                for t in range(L):
                    pending_ph2.append(make_ph2(b, soff, t, qts, Mf))

        emit_ph2(len(pending_ph2))

    nc.compile()
    return nc


# ---------------------------------------------------------------------------
# entry point
# ---------------------------------------------------------------------------

def _run(x, w_qkv, ln1_w, ln1_b, ln2_w, ln2_b, w_out, b_out, batch,
         num_graphs, n_cores=N_CORES, trace=False):
    x = np.ascontiguousarray(np.asarray(x, np.float32))
    counts, starts, core_graphs, Ls = _plan(batch, num_graphs, n_cores)
    per_core, slot_off = _pack_inputs(x, counts, starts, core_graphs, Ls, n_cores)
    T = sum(Ls)

    ln1_w = np.asarray(ln1_w, np.float32)
    ln1_b = np.asarray(ln1_b, np.float32)
    ln2_w = np.asarray(ln2_w, np.float32)
    ln2_b = np.asarray(ln2_b, np.float32)
    ln_general = not (
        np.all(ln1_w == 1.0) and np.all(ln1_b == 0.0)
        and np.all(ln2_w == 1.0) and np.all(ln2_b == 0.0)
    )

    bout_np = np.asarray(b_out, np.float32)
    bo_zero = bool(np.all(bout_np == 0.0))
    key = (T, tuple(Ls), n_cores, ln_general, bo_zero)
    nc = _PROGRAM_CACHE.get(key)
    if nc is None:
        nc = _build_program(T, tuple(Ls), n_cores, ln_general, bo_zero)
        _PROGRAM_CACHE[key] = nc

    wqkvT = np.ascontiguousarray(np.asarray(w_qkv, np.float32).T)
    woutT = np.ascontiguousarray(np.asarray(w_out, np.float32).T)
    bout = np.ascontiguousarray(np.asarray(b_out, np.float32))
    lnp = np.stack([ln1_w, ln1_b, ln2_w, ln2_b])
    # per-head column means of W_k | W_v: mean(k)[n,h] = x[n] @ wsum[:, h]
    wsum = np.ascontiguousarray(
        wqkvT[:, INNER:].reshape(DIM, 2, HEADS, DH).mean(-1).reshape(DIM, -1))

    in_maps = []
    for c in range(n_cores):
        xTp, qscv, kvmv = per_core[c]
        m = {"xT": xTp, "wqkvT": wqkvT, "wsum": wsum, "woutT": woutT,
             "bout": bout, "qsc": qscv, "kvm": kvmv}
        if ln_general:
            m["lnp"] = lnp
        in_maps.append(m)

    res = run_bass_kernel_spmd(nc, in_maps, list(range(n_cores)), trace=trace)

    N = x.shape[1]
    y = np.empty((B, N, DIM), np.float32)
    for c in range(n_cores):
        oc = res.results[c]["out"]
        for s, g in enumerate(core_graphs[c]):
            if g < 0 or counts[g] == 0:
                continue
            n0, ng = int(starts[g]), int(counts[g])
            off = int(slot_off[s]) * P
            y[:, n0:n0 + ng, :] = oc[:, off:off + ng, :]
    return y, res


def kernel(**inputs):
    trace = bool(os.environ.get("GALERKIN_TRACE"))
    y, _ = _run(
        inputs["x"], inputs["w_qkv"], inputs["ln1_w"], inputs["ln1_b"],
        inputs["ln2_w"], inputs["ln2_b"], inputs["w_out"], inputs["b_out"],
        inputs["batch"], inputs["num_graphs"], trace=trace,
    )
    return y


# revision 34
# speedup vs baseline: 2.8729x; 2.8729x over previous
"""Galerkin attention (ragged graph segments) on 8 Trainium2 NeuronCores.

Math (per reference):
  qkv = x @ w_qkv.T ; split q,k,v -> [B, H, N, DH]
  k, v  <- LayerNorm over DH (eps=1e-6, affine)
  per graph g (sorted contiguous segments of N): ktv[g] = k_g^T v_g
  out_n = (q_n / size(g(n))) @ ktv[g(n)]
  y = out @ w_out.T + b_out

Sharding: 32 graphs are bin-packed onto 8 cores x S slots; every core runs
the identical instruction stream (SPMD) over T = sum(L_s) 128-row tiles per
batch entry, where L_s is the max tile count of slot s across cores. Ragged
graph ends are zero-padded; padding is neutralized by folding a 0/1 mask
into the LN scale (a = mask/std) and the final per-node output scale.

v2 dataflow per 128-node tile (engines balanced, PE kept streaming):
  PE : qkv projection (f32r, 512-free), full-cross pair k^T v (fp32,
       128-free, single PSUM bank per slot), per-slot Mf = ktv @ w_out.T,
       phase-2 out = q^T.T @ Mf.
  Act: squares for LN variance, sqrt(var+eps), psum->sbuf copies, out scale.
  DVE: two multi-group tensor_reduce (sum k / sum k^2 per head; reversed
       inner stride defeats the AP contiguity merge), tiny fixups, and the
       broadcast LN multiply (per-node-head scale via stride-0 AP).
  Pool(GpSimd): broadcast LN add (SBUF only; GPSIMD cannot touch PSUM).
Phase 2 of slot s-1 is emitted interleaved into phase 1 of slot s so the
in-order PE queue always has ready matmuls while the LN chain drains.
"""

import os
import sys

if "/opt/trn_rl_repo" not in sys.path:
    sys.path.insert(0, "/opt/trn_rl_repo")

import numpy as np

import concourse.bacc as bacc
import concourse.bass as bass
import concourse.mybir as mybir
import concourse.tile as tile
from concourse.bass_utils import run_bass_kernel_spmd

P = 128
B = 2
DIM = 512
HEADS = 8
DH = 64
INNER = HEADS * DH          # 512
R = 3 * INNER               # 1536
NCH = DIM // P              # 4 contraction chunks
NPAIRS = HEADS // 2         # 4 head pairs
EPS = 1e-6
N_CORES = 8
GRP = 4                     # tiles per matmul group (512-node span)
KTV_LAG = 5                 # tiles of software pipelining before k^T v
F32 = mybir.dt.float32
F32R = mybir.dt.float32r
BF16 = mybir.dt.bfloat16

_PROGRAM_CACHE: dict = {}


def _revap(src):
    """View of `src` with the innermost (contiguous) dim reversed.

    Stats are permutation-invariant; the negative stride stops the AP
    optimizer from merging the per-head groups into one flat run, which
    would turn a multi-group tensor_reduce into a single global one.
    """
    inner = list(src.ap[-1])
    assert inner[0] == 1 and inner[1] == DH
    return bass.AP(
        tensor=src.tensor,
        offset=src.offset + (DH - 1),
        ap=[list(d) for d in src.ap[:-1]] + [[-1, DH]],
    )


# ---------------------------------------------------------------------------
# host-side planning
# ---------------------------------------------------------------------------

def _plan(batch, num_graphs, n_cores):
    """Assign graphs to (core, slot) and compute the uniform slot widths."""
    batch = np.asarray(batch).astype(np.int64)
    G = int(num_graphs)
    counts = np.bincount(batch, minlength=G)[:G].astype(np.int64)
    starts = np.concatenate([[0], np.cumsum(counts)[:-1]])
    tiles_g = (counts + P - 1) // P

    # SPMD: every core executes the same T = sum_s max_c tiles, so only the
    # per-slot maxima matter. Sorting by size and filling slot s with ranks
    # [s*n_cores, (s+1)*n_cores) minimizes each slot's max simultaneously.
    S = (G + n_cores - 1) // n_cores
    order = list(np.argsort(-tiles_g, kind="stable")) + [-1] * (S * n_cores - G)
    core_graphs = [[] for _ in range(n_cores)]
    for s in range(S):
        for c in range(n_cores):
            core_graphs[c].append(int(order[s * n_cores + c]))

    Ls = []
    for s in range(S):
        L = max(
            int(tiles_g[core_graphs[c][s]]) if core_graphs[c][s] >= 0 else 0
            for c in range(n_cores)
        )
        Ls.append(max(L, 1))
    return counts, starts, core_graphs, Ls


def _pack_inputs(x, counts, starts, core_graphs, Ls, n_cores):
    T = sum(Ls)
    slot_off = np.concatenate([[0], np.cumsum(Ls)[:-1]])
    xT = np.ascontiguousarray(np.transpose(x, (0, 2, 1)))  # [B, DIM, N]
    per_core = []
    for c in range(n_cores):
        xTp = np.zeros((B, DIM, T * P), np.float32)
        qsc = np.zeros((T * P,), np.float32)
        kvm = np.zeros((T * P,), np.float32)
        for s, g in enumerate(core_graphs[c]):
            if g < 0 or counts[g] == 0:
                continue
            n0, ng = int(starts[g]), int(counts[g])
            off = int(slot_off[s]) * P
            xTp[:, :, off:off + ng] = xT[:, :, n0:n0 + ng]
            qsc[off:off + ng] = 1.0 / ng
            kvm[off:off + ng] = 1.0
        per_core.append((xTp, qsc, kvm))
    return per_core, slot_off


# ---------------------------------------------------------------------------
# device program
# ---------------------------------------------------------------------------

def _build_program(T, Ls, n_cores, ln_general, bo_zero=False):
    from contextlib import ExitStack

    nc = bacc.Bacc("TRN2", target_bir_lowering=False, debug=False,
                   num_devices=n_cores)

    xT = nc.dram_tensor("xT", [B, DIM, T * P], F32R, kind="ExternalInput")
    wq = nc.dram_tensor("wqkvT", [DIM, R], F32R, kind="ExternalInput")
    ws = nc.dram_tensor("wsum", [DIM, 2 * HEADS], F32R, kind="ExternalInput")
    wo = nc.dram_tensor("woutT", [INNER, DIM], F32R, kind="ExternalInput")
    bo = nc.dram_tensor("bout", [DIM], F32, kind="ExternalInput")
    qsc = nc.dram_tensor("qsc", [T * P], F32, kind="ExternalInput")
    kvm = nc.dram_tensor("kvm", [T * P], F32, kind="ExternalInput")
    if ln_general:
        lnp = nc.dram_tensor("lnp", [4, DH], F32, kind="ExternalInput")
    out = nc.dram_tensor("out", [B, T * P, DIM], F32, kind="ExternalOutput")

    slot_off = [0]
    for L in Ls[:-1]:
        slot_off.append(slot_off[-1] + L)

    Sqrt = mybir.ActivationFunctionType.Sqrt
    Square = mybir.ActivationFunctionType.Square
    mult = mybir.AluOpType.mult
    add = mybir.AluOpType.add
    sub = mybir.AluOpType.subtract
    X = mybir.AxisListType.X

    with ExitStack() as ctx:
        tc = ctx.enter_context(tile.TileContext(nc))
        const = ctx.enter_context(tc.tile_pool(name="const", bufs=1))

        xpool = ctx.enter_context(tc.tile_pool(name="xp", bufs=3))
        xTr = xT.ap()

        def xt_dma(b, n0, GW):
            t = xpool.tile([P, NCH, GW], F32R, name="xt", tag="xt")
            nc.sync.dma_start(
                out=t[:],
                in_=xTr[b].rearrange("(k c) n -> c k n", c=P)[:, :, n0:n0 + GW],
            )
            return t

        # first x tile group + W chunk 0 land before the bulk constants so
        # the PE pipeline starts ~20us earlier
        xt_first = xt_dma(0, 0, min(GRP, Ls[0]) * P)
        WQ = [const.tile([P, R], F32R, name=f"WQ{k}", tag=f"WQ{k}")
              for k in range(NCH)]
        wqr = wq.ap().rearrange("(k c) r -> k c r", c=P)
        for k in range(NCH):
            nc.sync.dma_start(out=WQ[k][:], in_=wqr[k])
        WS = const.tile([P, NCH, 2 * HEADS], F32R, tag="WS")
        nc.sync.dma_start(out=WS[:], in_=ws.ap().rearrange("(k c) r -> c k r", c=P))
        WO = const.tile([P, NCH, DIM], F32R, tag="WO")
        nc.sync.dma_start(out=WO[:], in_=wo.ap().rearrange("(k c) d -> c k d", c=P))
        BO = const.tile([P, DIM], F32, tag="BO")
        nc.sync.dma_start(out=BO[:], in_=bo.ap().partition_broadcast(P))
        QS = const.tile([P, T], F32, tag="QS")
        nc.sync.dma_start(out=QS[:], in_=qsc.ap().rearrange("(t p) -> p t", p=P))
        KM = const.tile([P, T], F32, tag="KM")
        nc.sync.dma_start(out=KM[:], in_=kvm.ap().rearrange("(t p) -> p t", p=P))
        EPSC = const.tile([P, 1], F32, tag="EPSC")
        nc.vector.memset(EPSC[:], EPS)
        if ln_general:
            LNP = const.tile([P, 4, DH], F32, tag="LNP")
            nc.sync.dma_start(out=LNP[:], in_=lnp.ap().partition_broadcast(P))

        sqpool = ctx.enter_context(tc.tile_pool(name="sqp", bufs=2))
        klvlp = ctx.enter_context(tc.tile_pool(name="klvlp", bufs=KTV_LAG + 2))
        stat = ctx.enter_context(tc.tile_pool(name="stat", bufs=27))
        qstash = ctx.enter_context(tc.tile_pool(name="qstash", bufs=3 * NPAIRS))
        bdsb = ctx.enter_context(tc.tile_pool(name="bd", bufs=2))
        mfsb = ctx.enter_context(tc.tile_pool(name="mf", bufs=2))
        outsb = ctx.enter_context(tc.tile_pool(name="outsb", bufs=3))

        kvps = ctx.enter_context(tc.tile_pool(name="kvps", bufs=2, space="PSUM"))
        qtps = ctx.enter_context(tc.tile_pool(name="qtps", bufs=1, space="PSUM"))
        ktps = ctx.enter_context(tc.tile_pool(name="ktps", bufs=1, space="PSUM"))
        mips = ctx.enter_context(tc.tile_pool(name="mips", bufs=1, space="PSUM"))
        mups = ctx.enter_context(tc.tile_pool(name="mups", bufs=1, space="PSUM"))

        # phase-2 emitters for the previous slot, interleaved into the next
        # slot's phase 1 to keep the PE queue stocked with ready matmuls
        pending_ph2 = []

        def emit_ph2(k=1, pools=(None,)):
            for i in range(k):
                if pending_ph2:
                    pending_ph2.pop(0)(pools[i % len(pools)])

        def make_ph2(b, soff, t, qts, Mf):
            ti = soff + t

            def go(ps_pool=None):
                if ps_pool is None:
                    opsv = mips.tile([P, DIM], F32, name="ops", tag="mi")[:]
                else:
                    opsv = ps_pool.tile([P, 2, INNER], F32, name="ops",
                                        tag="kv")[:, 0, :]
                for p in range(NPAIRS):
                    nc.tensor.matmul(
                        opsv,
                        lhsT=qts[p][:, t * P:(t + 1) * P],
                        rhs=Mf[:, p, :],
                        start=(p == 0), stop=(p == NPAIRS - 1),
                    )
                ot = outsb.tile([P, DIM], F32, tag="ot")
                if bo_zero:
                    nc.scalar.mul(ot[:], opsv, QS[:, ti:ti + 1])
                else:
                    nc.vector.scalar_tensor_tensor(
                        ot[:], opsv, QS[:, ti:ti + 1], BO[:],
                        op0=mult, op1=add)
                nc.sync.dma_start(
                    out=out.ap()[b, ti * P:(ti + 1) * P, :], in_=ot[:])

            return go

        # global deferred k^T v queue: emitters stay KTV_LAG tiles behind and
        # carry over slot boundaries; a finalizer item per slot emits the
        # bd/Mf reduction and queues that slot's phase 2
        ktv_queue = []
        ktv_backlog = [0]

        def emit_ktv():
            while ktv_queue:
                item, is_real = ktv_queue.pop(0)
                item()
                if is_real:
                    ktv_backlog[0] -= 1
                    return

        for b in range(B):
            for s, L in enumerate(Ls):
                soff = slot_off[s]
                ktv = ktps.tile([P, NPAIRS, P], F32, tag="ktv")
                qts = [qstash.tile([P, L * P], F32R, name=f"qts{i}", tag="qstash")
                       for i in range(NPAIRS)]
                ngroups = (L + GRP - 1) // GRP

                for grp in range(ngroups):
                    gt0 = grp * GRP
                    gw = min(GRP, L - gt0)
                    GW = gw * P
                    n0 = (soff + gt0) * P

                    if b == 0 and s == 0 and grp == 0:
                        xt = xt_first
                    else:
                        xt = xt_dma(b, n0, GW)

                    # q^T: stationary = W_q pair block, moving = x^T
                    for p in range(NPAIRS):
                        qtp = qtps.tile([P, GW], F32, tag="qtp")
                        for k in range(NCH):
                            nc.tensor.matmul(
                                qtp[:],
                                lhsT=WQ[k][:, p * P:(p + 1) * P],
                                rhs=xt[:, k, :],
                                start=(k == 0), stop=(k == NCH - 1),
                            )
                        nc.scalar.copy(out=qts[p][:, gt0 * P:gt0 * P + GW],
                                       in_=qtp[:])
                        emit_ph2()

                    for tl in range(gw):
                        t = gt0 + tl
                        ti = soff + t  # global tile index (mask/scale column)

                        kv = kvps.tile([P, 2, INNER], F32, tag="kv")
                        mu_ps = mups.tile([P, 2, HEADS], F32, tag="mu_ps")
                        for k in range(NCH):
                            lx = xt[:, k, tl * P:(tl + 1) * P]
                            nc.tensor.matmul(
                                kv[:, 0, :], lhsT=lx,
                                rhs=WQ[k][:, INNER:2 * INNER],
                                start=(k == 0), stop=(k == NCH - 1))
                            nc.tensor.matmul(
                                kv[:, 1, :], lhsT=lx,
                                rhs=WQ[k][:, 2 * INNER:R],
                                start=(k == 0), stop=(k == NCH - 1))
                            # per-head means, same stationary: mu = x @ Wsum/64
                            nc.tensor.matmul(
                                mu_ps[:].rearrange("p t h -> p (t h)"),
                                lhsT=lx, rhs=WS[:, k, :],
                                start=(k == 0), stop=(k == NCH - 1))

                        kv4 = kv[:].rearrange("p t (h d) -> p t h d", h=HEADS)

                        # LN variance: squares on Act, per-head sums via one
                        # multi-group reduce; mean comes from the matmul above
                        sq = sqpool.tile([P, 2, HEADS, DH + 4], F32, tag="sq")
                        nc.scalar.activation(sq[:, :, :, 0:DH], kv4, Square)
                        mus = stat.tile([P, 2, HEADS], F32, tag="mus")
                        nc.vector.tensor_copy(mus[:], mu_ps[:])
                        msq = stat.tile([P, 2, HEADS], F32, tag="msq")
                        nc.vector.tensor_reduce(out=msq[:],
                                                in_=sq[:, :, :, 0:DH],
                                                axis=X, op=add)
                        D2 = stat.tile([P, 2, HEADS], F32, tag="D2")
                        nc.vector.tensor_tensor(D2[:], mus[:], mus[:], op=mult)
                        var = stat.tile([P, 2, HEADS], F32, tag="var")
                        nc.vector.scalar_tensor_tensor(
                            var[:], msq[:], 1.0 / DH, D2[:], op0=mult, op1=sub)
                        stdt = stat.tile([P, 2, HEADS], F32, tag="stdt")
                        nc.scalar.activation(stdt[:], var[:], Sqrt,
                                             bias=EPSC[:, 0:1])
                        rstd = stat.tile([P, 2, HEADS], F32, tag="rstd")
                        nc.vector.reciprocal(rstd[:], stdt[:])
                        # c = -mu * rstd. No pad mask needed: pad rows have
                        # x = 0, so k = v = mu = 0 and klvl = 0 follows.
                        cv = stat.tile([P, 2, HEADS], BF16, tag="cv")
                        nc.vector.scalar_tensor_tensor(
                            cv[:], mus[:], -1.0, rstd[:], op0=mult, op1=mult)

                        # apply: mult on DVE (PSUM read), add on GpSimd (SBUF)
                        # klvl in bf16: feeds the k^T v matmuls at 1 cycle/row
                        klvl = klvlp.tile([P, 2, HEADS, DH], BF16, tag="klvl")
                        nc.vector.tensor_tensor(
                            klvl[:], kv4,
                            rstd[:, :, :, None].broadcast_to([P, 2, HEADS, DH]),
                            op=mult)
                        nc.gpsimd.tensor_tensor(
                            klvl[:], klvl[:],
                            cv[:, :, :, None].broadcast_to([P, 2, HEADS, DH]),
                            op=add)
                        if ln_general:
                            bmk = stat.tile([P, 2, DH], F32, tag="bmk")
                            nc.vector.tensor_scalar(bmk[:, 0], LNP[:, 1],
                                                    KM[:, ti:ti + 1], None,
                                                    op0=mult)
                            nc.vector.tensor_scalar(bmk[:, 1], LNP[:, 3],
                                                    KM[:, ti:ti + 1], None,
                                                    op0=mult)
                            for half, wi in ((0, 0), (1, 2)):
                                nc.vector.tensor_tensor(
                                    klvl[:, half], klvl[:, half],
                                    LNP[:, wi, None, :].broadcast_to(
                                        [P, HEADS, DH]), op=mult)
                                nc.vector.tensor_tensor(
                                    klvl[:, half], klvl[:, half],
                                    bmk[:, half, None, :].broadcast_to(
                                        [P, HEADS, DH]), op=add)

                        # full-cross pair k^T v (transposed: lhsT = v side),
                        # deferred KTV_LAG tiles to hide the LN chain latency
                        def make_ktv(klvl=klvl, t=t, ktv=ktv, L=L):
                            def go():
                                klf = klvl[:, 0].rearrange("p h d -> p (h d)")
                                vlf = klvl[:, 1].rearrange("p h d -> p (h d)")
                                for p in range(NPAIRS):
                                    nc.tensor.matmul(
                                        ktv[:, p, :],
                                        lhsT=vlf[:, p * P:(p + 1) * P],
                                        rhs=klf[:, p * P:(p + 1) * P],
                                        start=(t == 0 and p == 0),
                                        stop=(t == L - 1 and p == NPAIRS - 1))
                            return go

                        ktv_queue.append((make_ktv(), True))
                        ktv_backlog[0] += 1
                        if ktv_backlog[0] > KTV_LAG:
                            emit_ktv()
                        emit_ph2()

                def make_finalize(b=b, soff=soff, L=L, ktv=ktv, qts=qts):
                    def go():
                        # block-diag (ktv_h)^T for Mf and phase 2
                        bd = bdsb.tile([P, NPAIRS, P], F32R, name="bd",
                                       tag="bd")
                        nc.gpsimd.memset(bd[:].bitcast(mybir.dt.uint32), 0)
                        for p in range(NPAIRS):
                            nc.vector.tensor_copy(bd[0:DH, p, 0:DH],
                                                  ktv[0:DH, p, 0:DH])
                            nc.vector.tensor_copy(bd[DH:P, p, DH:P],
                                                  ktv[DH:P, p, DH:P])
                        # Mf = blockdiag(ktv) @ w_out.T  [INNER-pair x DIM]
                        # (psum space borrowed from the kv pool)
                        Mf = mfsb.tile([P, NPAIRS, DIM], F32R, name="Mf",
                                       tag="Mf")
                        for half in range(2):
                            mfp = kvps.tile([P, 2, INNER], F32, name="mfp",
                                            tag="kv")
                            for i in range(2):
                                p = 2 * half + i
                                nc.tensor.matmul(
                                    mfp[:, i, :], lhsT=bd[:, p, :],
                                    rhs=WO[:, p, :], start=True, stop=True)
                            for i in range(2):
                                p = 2 * half + i
                                nc.scalar.copy(out=Mf[:, p, :],
                                               in_=mfp[:, i, :])
                        for t in range(L):
                            pending_ph2.append(make_ph2(b, soff, t, qts, Mf))
                    return go

                ktv_queue.append((make_finalize(), False))

        while ktv_queue:
            item, is_real = ktv_queue.pop(0)
            item()
        # final drain: nothing left to interleave, so spread the last slot's
        # phase 2 across the now-idle kv psum banks as well
        emit_ph2(len(pending_ph2), pools=(None, kvps))

    nc.compile()
    return nc


# ---------------------------------------------------------------------------
# entry point
# ---------------------------------------------------------------------------

def _run(x, w_qkv, ln1_w, ln1_b, ln2_w, ln2_b, w_out, b_out, batch,
         num_graphs, n_cores=N_CORES, trace=False):
    x = np.ascontiguousarray(np.asarray(x, np.float32))
    counts, starts, core_graphs, Ls = _plan(batch, num_graphs, n_cores)
    per_core, slot_off = _pack_inputs(x, counts, starts, core_graphs, Ls, n_cores)
    T = sum(Ls)

    ln1_w = np.asarray(ln1_w, np.float32)
    ln1_b = np.asarray(ln1_b, np.float32)
    ln2_w = np.asarray(ln2_w, np.float32)
    ln2_b = np.asarray(ln2_b, np.float32)
    ln_general = not (
        np.all(ln1_w == 1.0) and np.all(ln1_b == 0.0)
        and np.all(ln2_w == 1.0) and np.all(ln2_b == 0.0)
    )

    bout_np = np.asarray(b_out, np.float32)
    bo_zero = bool(np.all(bout_np == 0.0))
    key = (T, tuple(Ls), n_cores, ln_general, bo_zero)
    nc = _PROGRAM_CACHE.get(key)
    if nc is None:
        nc = _build_program(T, tuple(Ls), n_cores, ln_general, bo_zero)
        _PROGRAM_CACHE[key] = nc

    wqkvT = np.ascontiguousarray(np.asarray(w_qkv, np.float32).T)
    woutT = np.ascontiguousarray(np.asarray(w_out, np.float32).T)
    bout = np.ascontiguousarray(np.asarray(b_out, np.float32))
    lnp = np.stack([ln1_w, ln1_b, ln2_w, ln2_b])
    # per-head column means of W_k | W_v: mean(k)[n,h] = x[n] @ wsum[:, h]
    wsum = np.ascontiguousarray(
        wqkvT[:, INNER:].reshape(DIM, 2, HEADS, DH).mean(-1).reshape(DIM, -1))

    in_maps = []
    for c in range(n_cores):
        xTp, qscv, kvmv = per_core[c]
        m = {"xT": xTp, "wqkvT": wqkvT, "wsum": wsum, "woutT": woutT,
             "bout": bout, "qsc": qscv, "kvm": kvmv}
        if ln_general:
            m["lnp"] = lnp
        in_maps.append(m)

    res = run_bass_kernel_spmd(nc, in_maps, list(range(n_cores)), trace=trace)

    N = x.shape[1]
    y = np.empty((B, N, DIM), np.float32)
    for c in range(n_cores):
        oc = res.results[c]["out"]
        for s, g in enumerate(core_graphs[c]):
            if g < 0 or counts[g] == 0:
                continue
            n0, ng = int(starts[g]), int(counts[g])
            off = int(slot_off[s]) * P
            y[:, n0:n0 + ng, :] = oc[:, off:off + ng, :]
    return y, res


def kernel(**inputs):
    trace = bool(os.environ.get("GALERKIN_TRACE"))
    y, _ = _run(
        inputs["x"], inputs["w_qkv"], inputs["ln1_w"], inputs["ln1_b"],
        inputs["ln2_w"], inputs["ln2_b"], inputs["w_out"], inputs["b_out"],
        inputs["batch"], inputs["num_graphs"], trace=trace,
    )
    return y


# revision 36
# speedup vs baseline: 2.8785x; 1.0019x over previous
"""Galerkin attention (ragged graph segments) on 8 Trainium2 NeuronCores.

Math (per reference):
  qkv = x @ w_qkv.T ; split q,k,v -> [B, H, N, DH]
  k, v  <- LayerNorm over DH (eps=1e-6, affine)
  per graph g (sorted contiguous segments of N): ktv[g] = k_g^T v_g
  out_n = (q_n / size(g(n))) @ ktv[g(n)]
  y = out @ w_out.T + b_out

Sharding: 32 graphs are assigned to 8 cores x S slots by sorted rank (only
the per-slot maxima matter under SPMD); every core runs the identical
instruction stream over T = sum(L_s) 128-row tiles per batch entry. Ragged
graph ends are zero-padded; zero rows yield mu = k = v = 0 so LN emits
exact zeros without masking, and the per-node output scale (1/graph size,
zero on pads) handles the q side.

Dataflow per 128-node tile (engines balanced, PE kept streaming):
  PE : qkv projection (f32r, 512-free), per-head LN means via a folded
       wsum matmul, full-cross pair k^T v (bf16, 128-free, one PSUM bank
       per slot), per-slot Mf = blockdiag(ktv) @ w_out.T, and phase 2
       out = (q^T)^T @ Mf (the out-projection is pre-folded into Mf).
  Act: squares for LN variance, sqrt(var+eps), psum->sbuf copies, out scale.
  DVE: one multi-group tensor_reduce (sum k^2 per head), tiny fixups, and
       the broadcast LN multiply (per-node-head scale via stride-0 AP).
  Pool(GpSimd): broadcast LN add (SBUF only; GPSIMD cannot touch PSUM).
The k^T v matmuls trail the projection by KTV_LAG tiles (carried across
slot boundaries), and phase 2 of slot s-1 is emitted interleaved into
phase 1 of slot s, so the in-order PE queue always has ready matmuls
while the LN chain drains.
"""

import os
import sys

if "/opt/trn_rl_repo" not in sys.path:
    sys.path.insert(0, "/opt/trn_rl_repo")

import numpy as np

import concourse.bacc as bacc
import concourse.bass as bass
import concourse.mybir as mybir
import concourse.tile as tile
from concourse.bass_utils import run_bass_kernel_spmd

P = 128
B = 2
DIM = 512
HEADS = 8
DH = 64
INNER = HEADS * DH          # 512
R = 3 * INNER               # 1536
NCH = DIM // P              # 4 contraction chunks
NPAIRS = HEADS // 2         # 4 head pairs
EPS = 1e-6
N_CORES = 8
GRP = 4                     # tiles per matmul group (512-node span)
KTV_LAG = 4                 # tiles of software pipelining before k^T v
F32 = mybir.dt.float32
F32R = mybir.dt.float32r
BF16 = mybir.dt.bfloat16

_PROGRAM_CACHE: dict = {}


def _revap(src):
    """View of `src` with the innermost (contiguous) dim reversed.

    Stats are permutation-invariant; the negative stride stops the AP
    optimizer from merging the per-head groups into one flat run, which
    would turn a multi-group tensor_reduce into a single global one.
    """
    inner = list(src.ap[-1])
    assert inner[0] == 1 and inner[1] == DH
    return bass.AP(
        tensor=src.tensor,
        offset=src.offset + (DH - 1),
        ap=[list(d) for d in src.ap[:-1]] + [[-1, DH]],
    )


# ---------------------------------------------------------------------------
# host-side planning
# ---------------------------------------------------------------------------

def _plan(batch, num_graphs, n_cores):
    """Assign graphs to (core, slot) and compute the uniform slot widths."""
    batch = np.asarray(batch).astype(np.int64)
    G = int(num_graphs)
    counts = np.bincount(batch, minlength=G)[:G].astype(np.int64)
    starts = np.concatenate([[0], np.cumsum(counts)[:-1]])
    tiles_g = (counts + P - 1) // P

    # SPMD: every core executes the same T = sum_s max_c tiles, so only the
    # per-slot maxima matter. Sorting by size and filling slot s with ranks
    # [s*n_cores, (s+1)*n_cores) minimizes each slot's max simultaneously.
    S = (G + n_cores - 1) // n_cores
    order = list(np.argsort(-tiles_g, kind="stable")) + [-1] * (S * n_cores - G)
    core_graphs = [[] for _ in range(n_cores)]
    for s in range(S):
        for c in range(n_cores):
            core_graphs[c].append(int(order[s * n_cores + c]))

    Ls = []
    for s in range(S):
        L = max(
            int(tiles_g[core_graphs[c][s]]) if core_graphs[c][s] >= 0 else 0
            for c in range(n_cores)
        )
        Ls.append(max(L, 1))
    return counts, starts, core_graphs, Ls


def _pack_inputs(x, counts, starts, core_graphs, Ls, n_cores):
    T = sum(Ls)
    slot_off = np.concatenate([[0], np.cumsum(Ls)[:-1]])
    xT = np.ascontiguousarray(np.transpose(x, (0, 2, 1)))  # [B, DIM, N]
    per_core = []
    for c in range(n_cores):
        xTp = np.zeros((B, DIM, T * P), np.float32)
        qsc = np.zeros((T * P,), np.float32)
        kvm = np.zeros((T * P,), np.float32)
        for s, g in enumerate(core_graphs[c]):
            if g < 0 or counts[g] == 0:
                continue
            n0, ng = int(starts[g]), int(counts[g])
            off = int(slot_off[s]) * P
            xTp[:, :, off:off + ng] = xT[:, :, n0:n0 + ng]
            qsc[off:off + ng] = 1.0 / ng
            kvm[off:off + ng] = 1.0
        per_core.append((xTp, qsc, kvm))
    return per_core, slot_off


# ---------------------------------------------------------------------------
# device program
# ---------------------------------------------------------------------------

def _build_program(T, Ls, n_cores, ln_general, bo_zero=False):
    from contextlib import ExitStack

    nc = bacc.Bacc("TRN2", target_bir_lowering=False, debug=False,
                   num_devices=n_cores)

    xT = nc.dram_tensor("xT", [B, DIM, T * P], F32R, kind="ExternalInput")
    wq = nc.dram_tensor("wqkvT", [DIM, R], F32R, kind="ExternalInput")
    ws = nc.dram_tensor("wsum", [DIM, 2 * HEADS], F32R, kind="ExternalInput")
    wo = nc.dram_tensor("woutT", [INNER, DIM], F32R, kind="ExternalInput")
    bo = nc.dram_tensor("bout", [DIM], F32, kind="ExternalInput")
    qsc = nc.dram_tensor("qsc", [T * P], F32, kind="ExternalInput")
    kvm = nc.dram_tensor("kvm", [T * P], F32, kind="ExternalInput")
    if ln_general:
        lnp = nc.dram_tensor("lnp", [4, DH], F32, kind="ExternalInput")
    out = nc.dram_tensor("out", [B, T * P, DIM], F32, kind="ExternalOutput")

    slot_off = [0]
    for L in Ls[:-1]:
        slot_off.append(slot_off[-1] + L)

    Sqrt = mybir.ActivationFunctionType.Sqrt
    Square = mybir.ActivationFunctionType.Square
    mult = mybir.AluOpType.mult
    add = mybir.AluOpType.add
    sub = mybir.AluOpType.subtract
    X = mybir.AxisListType.X

    with ExitStack() as ctx:
        tc = ctx.enter_context(tile.TileContext(nc))
        const = ctx.enter_context(tc.tile_pool(name="const", bufs=1))

        xpool = ctx.enter_context(tc.tile_pool(name="xp", bufs=3))
        xTr = xT.ap()

        def xt_dma(b, n0, GW):
            t = xpool.tile([P, NCH, GW], F32R, name="xt", tag="xt")
            nc.sync.dma_start(
                out=t[:],
                in_=xTr[b].rearrange("(k c) n -> c k n", c=P)[:, :, n0:n0 + GW],
            )
            return t

        # first x tile group + W chunk 0 land before the bulk constants so
        # the PE pipeline starts ~20us earlier
        xt_first = xt_dma(0, 0, min(GRP, Ls[0]) * P)
        WQ = [const.tile([P, R], F32R, name=f"WQ{k}", tag=f"WQ{k}")
              for k in range(NCH)]
        wqr = wq.ap().rearrange("(k c) r -> k c r", c=P)
        for k in range(NCH):
            nc.sync.dma_start(out=WQ[k][:], in_=wqr[k])
        WS = const.tile([P, NCH, 2 * HEADS], F32R, tag="WS")
        nc.sync.dma_start(out=WS[:], in_=ws.ap().rearrange("(k c) r -> c k r", c=P))
        WO = const.tile([P, NCH, DIM], F32R, tag="WO")
        nc.sync.dma_start(out=WO[:], in_=wo.ap().rearrange("(k c) d -> c k d", c=P))
        BO = const.tile([P, DIM], F32, tag="BO")
        nc.sync.dma_start(out=BO[:], in_=bo.ap().partition_broadcast(P))
        QS = const.tile([P, T], F32, tag="QS")
        nc.sync.dma_start(out=QS[:], in_=qsc.ap().rearrange("(t p) -> p t", p=P))
        KM = const.tile([P, T], F32, tag="KM")
        nc.sync.dma_start(out=KM[:], in_=kvm.ap().rearrange("(t p) -> p t", p=P))
        EPSC = const.tile([P, 1], F32, tag="EPSC")
        nc.vector.memset(EPSC[:], EPS)
        if ln_general:
            LNP = const.tile([P, 4, DH], F32, tag="LNP")
            nc.sync.dma_start(out=LNP[:], in_=lnp.ap().partition_broadcast(P))

        sqpool = ctx.enter_context(tc.tile_pool(name="sqp", bufs=2))
        klvlp = ctx.enter_context(tc.tile_pool(name="klvlp", bufs=KTV_LAG + 2))
        stat = ctx.enter_context(tc.tile_pool(name="stat", bufs=27))
        qstash = ctx.enter_context(tc.tile_pool(name="qstash", bufs=3 * NPAIRS))
        bdsb = ctx.enter_context(tc.tile_pool(name="bd", bufs=2))
        mfsb = ctx.enter_context(tc.tile_pool(name="mf", bufs=2))
        outsb = ctx.enter_context(tc.tile_pool(name="outsb", bufs=3))

        kvps = ctx.enter_context(tc.tile_pool(name="kvps", bufs=2, space="PSUM"))
        qtps = ctx.enter_context(tc.tile_pool(name="qtps", bufs=1, space="PSUM"))
        ktps = ctx.enter_context(tc.tile_pool(name="ktps", bufs=1, space="PSUM"))
        mips = ctx.enter_context(tc.tile_pool(name="mips", bufs=1, space="PSUM"))
        mups = ctx.enter_context(tc.tile_pool(name="mups", bufs=1, space="PSUM"))

        # phase-2 emitters for the previous slot, interleaved into the next
        # slot's phase 1 to keep the PE queue stocked with ready matmuls
        pending_ph2 = []

        def emit_ph2(k=1, pools=(None,)):
            for i in range(k):
                if pending_ph2:
                    pending_ph2.pop(0)(pools[i % len(pools)])

        def make_ph2(b, soff, t, qts, Mf):
            ti = soff + t

            def go(ps_pool=None):
                if ps_pool is None:
                    opsv = mips.tile([P, DIM], F32, name="ops", tag="mi")[:]
                else:
                    opsv = ps_pool.tile([P, 2, INNER], F32, name="ops",
                                        tag="kv")[:, 0, :]
                for p in range(NPAIRS):
                    nc.tensor.matmul(
                        opsv,
                        lhsT=qts[p][:, t * P:(t + 1) * P],
                        rhs=Mf[:, p, :],
                        start=(p == 0), stop=(p == NPAIRS - 1),
                    )
                ot = outsb.tile([P, DIM], F32, tag="ot")
                if bo_zero:
                    nc.scalar.mul(ot[:], opsv, QS[:, ti:ti + 1])
                else:
                    nc.vector.scalar_tensor_tensor(
                        ot[:], opsv, QS[:, ti:ti + 1], BO[:],
                        op0=mult, op1=add)
                nc.sync.dma_start(
                    out=out.ap()[b, ti * P:(ti + 1) * P, :], in_=ot[:])

            return go

        # global deferred k^T v queue: emitters stay KTV_LAG tiles behind and
        # carry over slot boundaries; a finalizer item per slot emits the
        # bd/Mf reduction and queues that slot's phase 2
        ktv_queue = []
        ktv_backlog = [0]

        def emit_ktv():
            while ktv_queue:
                item, is_real = ktv_queue.pop(0)
                item()
                if is_real:
                    ktv_backlog[0] -= 1
                    return

        for b in range(B):
            for s, L in enumerate(Ls):
                soff = slot_off[s]
                ktv = ktps.tile([P, NPAIRS, P], F32, tag="ktv")
                qts = [qstash.tile([P, L * P], F32R, name=f"qts{i}", tag="qstash")
                       for i in range(NPAIRS)]
                ngroups = (L + GRP - 1) // GRP

                for grp in range(ngroups):
                    gt0 = grp * GRP
                    gw = min(GRP, L - gt0)
                    GW = gw * P
                    n0 = (soff + gt0) * P

                    if b == 0 and s == 0 and grp == 0:
                        xt = xt_first
                    else:
                        xt = xt_dma(b, n0, GW)

                    # q^T: stationary = W_q pair block, moving = x^T
                    for p in range(NPAIRS):
                        qtp = qtps.tile([P, GW], F32, tag="qtp")
                        for k in range(NCH):
                            nc.tensor.matmul(
                                qtp[:],
                                lhsT=WQ[k][:, p * P:(p + 1) * P],
                                rhs=xt[:, k, :],
                                start=(k == 0), stop=(k == NCH - 1),
                            )
                        nc.scalar.copy(out=qts[p][:, gt0 * P:gt0 * P + GW],
                                       in_=qtp[:])
                        emit_ph2()

                    for tl in range(gw):
                        t = gt0 + tl
                        ti = soff + t  # global tile index (mask/scale column)

                        kv = kvps.tile([P, 2, INNER], F32, tag="kv")
                        mu_ps = mups.tile([P, 2, HEADS], F32, tag="mu_ps")
                        for k in range(NCH):
                            lx = xt[:, k, tl * P:(tl + 1) * P]
                            nc.tensor.matmul(
                                kv[:, 0, :], lhsT=lx,
                                rhs=WQ[k][:, INNER:2 * INNER],
                                start=(k == 0), stop=(k == NCH - 1))
                            nc.tensor.matmul(
                                kv[:, 1, :], lhsT=lx,
                                rhs=WQ[k][:, 2 * INNER:R],
                                start=(k == 0), stop=(k == NCH - 1))
                            # per-head means, same stationary: mu = x @ Wsum/64
                            nc.tensor.matmul(
                                mu_ps[:].rearrange("p t h -> p (t h)"),
                                lhsT=lx, rhs=WS[:, k, :],
                                start=(k == 0), stop=(k == NCH - 1))

                        kv4 = kv[:].rearrange("p t (h d) -> p t h d", h=HEADS)

                        # LN variance: squares on Act, per-head sums via one
                        # multi-group reduce; mean comes from the matmul above
                        sq = sqpool.tile([P, 2, HEADS, DH + 4], F32, tag="sq")
                        nc.scalar.activation(sq[:, :, :, 0:DH], kv4, Square)
                        mus = stat.tile([P, 2, HEADS], F32, tag="mus")
                        nc.vector.tensor_copy(mus[:], mu_ps[:])
                        msq = stat.tile([P, 2, HEADS], F32, tag="msq")
                        nc.vector.tensor_reduce(out=msq[:],
                                                in_=sq[:, :, :, 0:DH],
                                                axis=X, op=add)
                        D2 = stat.tile([P, 2, HEADS], F32, tag="D2")
                        nc.vector.tensor_tensor(D2[:], mus[:], mus[:], op=mult)
                        var = stat.tile([P, 2, HEADS], F32, tag="var")
                        nc.vector.scalar_tensor_tensor(
                            var[:], msq[:], 1.0 / DH, D2[:], op0=mult, op1=sub)
                        stdt = stat.tile([P, 2, HEADS], F32, tag="stdt")
                        nc.scalar.activation(stdt[:], var[:], Sqrt,
                                             bias=EPSC[:, 0:1])
                        rstd = stat.tile([P, 2, HEADS], F32, tag="rstd")
                        nc.vector.reciprocal(rstd[:], stdt[:])
                        # c = -mu * rstd. No pad mask needed: pad rows have
                        # x = 0, so k = v = mu = 0 and klvl = 0 follows.
                        cv = stat.tile([P, 2, HEADS], BF16, tag="cv")
                        nc.vector.scalar_tensor_tensor(
                            cv[:], mus[:], -1.0, rstd[:], op0=mult, op1=mult)

                        # apply: mult on DVE (PSUM read), add on GpSimd (SBUF)
                        # klvl in bf16: feeds the k^T v matmuls at 1 cycle/row
                        klvl = klvlp.tile([P, 2, HEADS, DH], BF16, tag="klvl")
                        nc.vector.tensor_tensor(
                            klvl[:], kv4,
                            rstd[:, :, :, None].broadcast_to([P, 2, HEADS, DH]),
                            op=mult)
                        nc.gpsimd.tensor_tensor(
                            klvl[:], klvl[:],
                            cv[:, :, :, None].broadcast_to([P, 2, HEADS, DH]),
                            op=add)
                        if ln_general:
                            bmk = stat.tile([P, 2, DH], F32, tag="bmk")
                            nc.vector.tensor_scalar(bmk[:, 0], LNP[:, 1],
                                                    KM[:, ti:ti + 1], None,
                                                    op0=mult)
                            nc.vector.tensor_scalar(bmk[:, 1], LNP[:, 3],
                                                    KM[:, ti:ti + 1], None,
                                                    op0=mult)
                            for half, wi in ((0, 0), (1, 2)):
                                nc.vector.tensor_tensor(
                                    klvl[:, half], klvl[:, half],
                                    LNP[:, wi, None, :].broadcast_to(
                                        [P, HEADS, DH]), op=mult)
                                nc.vector.tensor_tensor(
                                    klvl[:, half], klvl[:, half],
                                    bmk[:, half, None, :].broadcast_to(
                                        [P, HEADS, DH]), op=add)

                        # full-cross pair k^T v (transposed: lhsT = v side),
                        # deferred KTV_LAG tiles to hide the LN chain latency
                        def make_ktv(klvl=klvl, t=t, ktv=ktv, L=L):
                            def go():
                                klf = klvl[:, 0].rearrange("p h d -> p (h d)")
                                vlf = klvl[:, 1].rearrange("p h d -> p (h d)")
                                for p in range(NPAIRS):
                                    nc.tensor.matmul(
                                        ktv[:, p, :],
                                        lhsT=vlf[:, p * P:(p + 1) * P],
                                        rhs=klf[:, p * P:(p + 1) * P],
                                        start=(t == 0 and p == 0),
                                        stop=(t == L - 1 and p == NPAIRS - 1))
                            return go

                        ktv_queue.append((make_ktv(), True))
                        ktv_backlog[0] += 1
                        if ktv_backlog[0] > KTV_LAG:
                            emit_ktv()
                        emit_ph2()

                def make_finalize(b=b, soff=soff, L=L, ktv=ktv, qts=qts):
                    def go():
                        # block-diag (ktv_h)^T for Mf and phase 2
                        bd = bdsb.tile([P, NPAIRS, P], F32R, name="bd",
                                       tag="bd")
                        nc.gpsimd.memset(bd[:].bitcast(mybir.dt.uint32), 0)
                        for p in range(NPAIRS):
                            nc.vector.tensor_copy(bd[0:DH, p, 0:DH],
                                                  ktv[0:DH, p, 0:DH])
                            nc.vector.tensor_copy(bd[DH:P, p, DH:P],
                                                  ktv[DH:P, p, DH:P])
                        # Mf = blockdiag(ktv) @ w_out.T  [INNER-pair x DIM]
                        # (psum space borrowed from the kv pool)
                        Mf = mfsb.tile([P, NPAIRS, DIM], F32R, name="Mf",
                                       tag="Mf")
                        for half in range(2):
                            mfp = kvps.tile([P, 2, INNER], F32, name="mfp",
                                            tag="kv")
                            for i in range(2):
                                p = 2 * half + i
                                nc.tensor.matmul(
                                    mfp[:, i, :], lhsT=bd[:, p, :],
                                    rhs=WO[:, p, :], start=True, stop=True)
                            for i in range(2):
                                p = 2 * half + i
                                nc.scalar.copy(out=Mf[:, p, :],
                                               in_=mfp[:, i, :])
                        for t in range(L):
                            pending_ph2.append(make_ph2(b, soff, t, qts, Mf))
                    return go

                ktv_queue.append((make_finalize(), False))

        while ktv_queue:
            item, is_real = ktv_queue.pop(0)
            item()
        # final drain: nothing left to interleave, so spread the last slot's
        # phase 2 across the now-idle kv psum banks as well
        emit_ph2(len(pending_ph2), pools=(None, kvps))

    nc.compile()
    return nc


# ---------------------------------------------------------------------------
# entry point
# ---------------------------------------------------------------------------

def _run(x, w_qkv, ln1_w, ln1_b, ln2_w, ln2_b, w_out, b_out, batch,
         num_graphs, n_cores=N_CORES, trace=False):
    x = np.ascontiguousarray(np.asarray(x, np.float32))
    counts, starts, core_graphs, Ls = _plan(batch, num_graphs, n_cores)
    per_core, slot_off = _pack_inputs(x, counts, starts, core_graphs, Ls, n_cores)
    T = sum(Ls)

    ln1_w = np.asarray(ln1_w, np.float32)
    ln1_b = np.asarray(ln1_b, np.float32)
    ln2_w = np.asarray(ln2_w, np.float32)
    ln2_b = np.asarray(ln2_b, np.float32)
    ln_general = not (
        np.all(ln1_w == 1.0) and np.all(ln1_b == 0.0)
        and np.all(ln2_w == 1.0) and np.all(ln2_b == 0.0)
    )

    bout_np = np.asarray(b_out, np.float32)
    bo_zero = bool(np.all(bout_np == 0.0))
    key = (T, tuple(Ls), n_cores, ln_general, bo_zero)
    nc = _PROGRAM_CACHE.get(key)
    if nc is None:
        nc = _build_program(T, tuple(Ls), n_cores, ln_general, bo_zero)
        _PROGRAM_CACHE[key] = nc

    wqkvT = np.ascontiguousarray(np.asarray(w_qkv, np.float32).T)
    woutT = np.ascontiguousarray(np.asarray(w_out, np.float32).T)
    bout = np.ascontiguousarray(np.asarray(b_out, np.float32))
    lnp = np.stack([ln1_w, ln1_b, ln2_w, ln2_b])
    # per-head column means of W_k | W_v: mean(k)[n,h] = x[n] @ wsum[:, h]
    wsum = np.ascontiguousarray(
        wqkvT[:, INNER:].reshape(DIM, 2, HEADS, DH).mean(-1).reshape(DIM, -1))

    in_maps = []
    for c in range(n_cores):
        xTp, qscv, kvmv = per_core[c]
        m = {"xT": xTp, "wqkvT": wqkvT, "wsum": wsum, "woutT": woutT,
             "bout": bout, "qsc": qscv, "kvm": kvmv}
        if ln_general:
            m["lnp"] = lnp
        in_maps.append(m)

    res = run_bass_kernel_spmd(nc, in_maps, list(range(n_cores)), trace=trace)

    N = x.shape[1]
    y = np.empty((B, N, DIM), np.float32)
    for c in range(n_cores):
        oc = res.results[c]["out"]
        for s, g in enumerate(core_graphs[c]):
            if g < 0 or counts[g] == 0:
                continue
            n0, ng = int(starts[g]), int(counts[g])
            off = int(slot_off[s]) * P
            y[:, n0:n0 + ng, :] = oc[:, off:off + ng, :]
    return y, res


def kernel(**inputs):
    trace = bool(os.environ.get("GALERKIN_TRACE"))
    y, _ = _run(
        inputs["x"], inputs["w_qkv"], inputs["ln1_w"], inputs["ln1_b"],
        inputs["ln2_w"], inputs["ln2_b"], inputs["w_out"], inputs["b_out"],
        inputs["batch"], inputs["num_graphs"], trace=trace,
    )
    return y


# revision 39
# speedup vs baseline: 3.0010x; 1.0426x over previous
"""Galerkin attention (ragged graph segments) on 8 Trainium2 NeuronCores.

Math (per reference):
  qkv = x @ w_qkv.T ; split q,k,v -> [B, H, N, DH]
  k, v  <- LayerNorm over DH (eps=1e-6, affine)
  per graph g (sorted contiguous segments of N): ktv[g] = k_g^T v_g
  out_n = (q_n / size(g(n))) @ ktv[g(n)]
  y = out @ w_out.T + b_out

Sharding: 32 graphs are assigned to 8 cores x S slots by sorted rank (only
the per-slot maxima matter under SPMD); every core runs the identical
instruction stream over T = sum(L_s) 128-row tiles per batch entry. Ragged
graph ends are zero-padded; zero rows yield mu = k = v = 0 so LN emits
exact zeros without masking, and the per-node output scale (1/graph size,
zero on pads) handles the q side.

Dataflow per 128-node tile (engines balanced, PE kept streaming):
  PE : qkv projection (f32r, 512-free), per-head LN means via a folded
       wsum matmul, full-cross pair k^T v (bf16, 128-free, one PSUM bank
       per slot), per-slot Mf = blockdiag(ktv) @ w_out.T, and phase 2
       out = (q^T)^T @ Mf (the out-projection is pre-folded into Mf).
  Act: squares for LN variance, sqrt(var+eps), psum->sbuf copies, out scale.
  DVE: one multi-group tensor_reduce (sum k^2 per head), tiny fixups, and
       the broadcast LN multiply (per-node-head scale via stride-0 AP).
  Pool(GpSimd): broadcast LN add (SBUF only; GPSIMD cannot touch PSUM).
The k^T v matmuls trail the projection by KTV_LAG tiles (carried across
slot boundaries), and phase 2 of slot s-1 is emitted interleaved into
phase 1 of slot s, so the in-order PE queue always has ready matmuls
while the LN chain drains.
"""

import os
import sys

if "/opt/trn_rl_repo" not in sys.path:
    sys.path.insert(0, "/opt/trn_rl_repo")

import numpy as np

import concourse.bacc as bacc
import concourse.bass as bass
import concourse.mybir as mybir
import concourse.tile as tile
from concourse.bass_utils import run_bass_kernel_spmd

P = 128
B = 2
DIM = 512
HEADS = 8
DH = 64
INNER = HEADS * DH          # 512
R = 3 * INNER               # 1536
NCH = DIM // P              # 4 contraction chunks
NPAIRS = HEADS // 2         # 4 head pairs
EPS = 1e-6
N_CORES = 8
GRP = 4                     # tiles per matmul group (512-node span)
KTV_LAG = 4                 # tiles of software pipelining before k^T v
F32 = mybir.dt.float32
F32R = mybir.dt.float32r
BF16 = mybir.dt.bfloat16

_PROGRAM_CACHE: dict = {}


def _revap(src):
    """View of `src` with the innermost (contiguous) dim reversed.

    Stats are permutation-invariant; the negative stride stops the AP
    optimizer from merging the per-head groups into one flat run, which
    would turn a multi-group tensor_reduce into a single global one.
    """
    inner = list(src.ap[-1])
    assert inner[0] == 1 and inner[1] == DH
    return bass.AP(
        tensor=src.tensor,
        offset=src.offset + (DH - 1),
        ap=[list(d) for d in src.ap[:-1]] + [[-1, DH]],
    )


# ---------------------------------------------------------------------------
# host-side planning
# ---------------------------------------------------------------------------

def _plan(batch, num_graphs, n_cores):
    """Assign graphs to (core, slot) and compute the uniform slot widths."""
    batch = np.asarray(batch).astype(np.int64)
    G = int(num_graphs)
    counts = np.bincount(batch, minlength=G)[:G].astype(np.int64)
    starts = np.concatenate([[0], np.cumsum(counts)[:-1]])
    tiles_g = (counts + P - 1) // P

    # SPMD: every core executes the same T = sum_s max_c tiles, so only the
    # per-slot maxima matter. Sorting by size and filling slot s with ranks
    # [s*n_cores, (s+1)*n_cores) minimizes each slot's max simultaneously.
    S = (G + n_cores - 1) // n_cores
    order = list(np.argsort(-tiles_g, kind="stable")) + [-1] * (S * n_cores - G)
    core_graphs = [[] for _ in range(n_cores)]
    for s in range(S):
        for c in range(n_cores):
            core_graphs[c].append(int(order[s * n_cores + c]))

    Ls = []
    for s in range(S):
        L = max(
            int(tiles_g[core_graphs[c][s]]) if core_graphs[c][s] >= 0 else 0
            for c in range(n_cores)
        )
        Ls.append(max(L, 1))
    return counts, starts, core_graphs, Ls


def _pack_inputs(x, counts, starts, core_graphs, Ls, n_cores):
    T = sum(Ls)
    slot_off = np.concatenate([[0], np.cumsum(Ls)[:-1]])
    xT = np.ascontiguousarray(np.transpose(x, (0, 2, 1)))  # [B, DIM, N]
    per_core = []
    for c in range(n_cores):
        xTp = np.zeros((B, DIM, T * P), np.float32)
        qsc = np.zeros((T * P,), np.float32)
        kvm = np.zeros((T * P,), np.float32)
        for s, g in enumerate(core_graphs[c]):
            if g < 0 or counts[g] == 0:
                continue
            n0, ng = int(starts[g]), int(counts[g])
            off = int(slot_off[s]) * P
            xTp[:, :, off:off + ng] = xT[:, :, n0:n0 + ng]
            qsc[off:off + ng] = 1.0 / ng
            kvm[off:off + ng] = 1.0
        per_core.append((xTp, qsc, kvm))
    return per_core, slot_off


# ---------------------------------------------------------------------------
# device program
# ---------------------------------------------------------------------------

def _build_program(T, Ls, n_cores, ln_general, bo_zero=False):
    from contextlib import ExitStack

    nc = bacc.Bacc("TRN2", target_bir_lowering=False, debug=False,
                   num_devices=n_cores)

    xT = nc.dram_tensor("xT", [B, DIM, T * P], F32R, kind="ExternalInput")
    wq = nc.dram_tensor("wqkvT", [DIM, R], F32R, kind="ExternalInput")
    ws = nc.dram_tensor("wsum", [DIM, 2 * HEADS], F32R, kind="ExternalInput")
    wo = nc.dram_tensor("woutT", [INNER, DIM], F32R, kind="ExternalInput")
    bo = nc.dram_tensor("bout", [DIM], F32, kind="ExternalInput")
    qsc = nc.dram_tensor("qsc", [T * P], F32, kind="ExternalInput")
    kvm = nc.dram_tensor("kvm", [T * P], F32, kind="ExternalInput")
    if ln_general:
        lnp = nc.dram_tensor("lnp", [4, DH], F32, kind="ExternalInput")
    out = nc.dram_tensor("out", [B, T * P, DIM], F32, kind="ExternalOutput")

    slot_off = [0]
    for L in Ls[:-1]:
        slot_off.append(slot_off[-1] + L)

    Sqrt = mybir.ActivationFunctionType.Sqrt
    Square = mybir.ActivationFunctionType.Square
    mult = mybir.AluOpType.mult
    add = mybir.AluOpType.add
    sub = mybir.AluOpType.subtract
    X = mybir.AxisListType.X

    with ExitStack() as ctx:
        tc = ctx.enter_context(tile.TileContext(nc))
        const = ctx.enter_context(tc.tile_pool(name="const", bufs=1))

        xpool = ctx.enter_context(tc.tile_pool(name="xp", bufs=4))
        xTr = xT.ap()

        def xt_dma(b, n0, GW):
            t = xpool.tile([P, NCH, GW], F32R, name="xt", tag="xt")
            nc.sync.dma_start(
                out=t[:],
                in_=xTr[b].rearrange("(k c) n -> c k n", c=P)[:, :, n0:n0 + GW],
            )
            return t

        # first x tile group + W chunk 0 land before the bulk constants so
        # the PE pipeline starts ~20us earlier
        xt_first = xt_dma(0, 0, min(GRP, Ls[0]) * P)
        WQ = [const.tile([P, R], F32R, name=f"WQ{k}", tag=f"WQ{k}")
              for k in range(NCH)]
        wqr = wq.ap().rearrange("(k c) r -> k c r", c=P)
        for k in range(NCH):
            nc.sync.dma_start(out=WQ[k][:], in_=wqr[k])
        WS = const.tile([P, NCH, 2 * HEADS], F32R, tag="WS")
        nc.sync.dma_start(out=WS[:], in_=ws.ap().rearrange("(k c) r -> c k r", c=P))
        WO = const.tile([P, NCH, DIM], F32R, tag="WO")
        nc.sync.dma_start(out=WO[:], in_=wo.ap().rearrange("(k c) d -> c k d", c=P))
        BO = const.tile([P, DIM], F32, tag="BO")
        nc.sync.dma_start(out=BO[:], in_=bo.ap().partition_broadcast(P))
        QS = const.tile([P, T], F32, tag="QS")
        nc.sync.dma_start(out=QS[:], in_=qsc.ap().rearrange("(t p) -> p t", p=P))
        KM = const.tile([P, T], F32, tag="KM")
        nc.sync.dma_start(out=KM[:], in_=kvm.ap().rearrange("(t p) -> p t", p=P))
        EPSC = const.tile([P, 1], F32, tag="EPSC")
        nc.vector.memset(EPSC[:], EPS)
        if ln_general:
            LNP = const.tile([P, 4, DH], F32, tag="LNP")
            nc.sync.dma_start(out=LNP[:], in_=lnp.ap().partition_broadcast(P))

        sqpool = ctx.enter_context(tc.tile_pool(name="sqp", bufs=3))
        klvlp = ctx.enter_context(tc.tile_pool(name="klvlp", bufs=KTV_LAG + 2))
        stat = ctx.enter_context(tc.tile_pool(name="stat", bufs=27))
        qstash = ctx.enter_context(tc.tile_pool(name="qstash", bufs=3 * NPAIRS))
        bdsb = ctx.enter_context(tc.tile_pool(name="bd", bufs=2))
        mfsb = ctx.enter_context(tc.tile_pool(name="mf", bufs=2))
        outsb = ctx.enter_context(tc.tile_pool(name="outsb", bufs=4))

        kvps = ctx.enter_context(tc.tile_pool(name="kvps", bufs=2, space="PSUM"))
        qtps = ctx.enter_context(tc.tile_pool(name="qtps", bufs=1, space="PSUM"))
        ktps = ctx.enter_context(tc.tile_pool(name="ktps", bufs=1, space="PSUM"))
        mips = ctx.enter_context(tc.tile_pool(name="mips", bufs=1, space="PSUM"))
        mups = ctx.enter_context(tc.tile_pool(name="mups", bufs=1, space="PSUM"))

        # phase-2 emitters for the previous slot, interleaved into the next
        # slot's phase 1 to keep the PE queue stocked with ready matmuls
        pending_ph2 = []

        def emit_ph2(k=1, pools=(None,)):
            for i in range(k):
                if pending_ph2:
                    pending_ph2.pop(0)(pools[i % len(pools)])

        def make_ph2(b, soff, t, qts, Mf):
            ti = soff + t

            def go(ps_pool=None):
                if ps_pool is None:
                    opsv = mips.tile([P, DIM], F32, name="ops", tag="mi")[:]
                else:
                    opsv = ps_pool.tile([P, 2, INNER], F32, name="ops",
                                        tag="kv")[:, 0, :]
                for p in range(NPAIRS):
                    nc.tensor.matmul(
                        opsv,
                        lhsT=qts[p][:, t * P:(t + 1) * P],
                        rhs=Mf[:, p, :],
                        start=(p == 0), stop=(p == NPAIRS - 1),
                    )
                ot = outsb.tile([P, DIM], F32, tag="ot")
                if bo_zero:
                    nc.scalar.mul(ot[:], opsv, QS[:, ti:ti + 1])
                else:
                    nc.vector.scalar_tensor_tensor(
                        ot[:], opsv, QS[:, ti:ti + 1], BO[:],
                        op0=mult, op1=add)
                nc.sync.dma_start(
                    out=out.ap()[b, ti * P:(ti + 1) * P, :], in_=ot[:])

            return go

        # global deferred k^T v queue: emitters stay KTV_LAG tiles behind and
        # carry over slot boundaries; a finalizer item per slot emits the
        # bd/Mf reduction and queues that slot's phase 2
        ktv_queue = []
        ktv_backlog = [0]

        def emit_ktv():
            while ktv_queue:
                item, is_real = ktv_queue.pop(0)
                item()
                if is_real:
                    ktv_backlog[0] -= 1
                    return

        for b in range(B):
            for s, L in enumerate(Ls):
                soff = slot_off[s]
                ktv = ktps.tile([P, NPAIRS, P], F32, tag="ktv")
                qts = [qstash.tile([P, L * P], F32R, name=f"qts{i}", tag="qstash")
                       for i in range(NPAIRS)]
                ngroups = (L + GRP - 1) // GRP

                for grp in range(ngroups):
                    gt0 = grp * GRP
                    gw = min(GRP, L - gt0)
                    GW = gw * P
                    n0 = (soff + gt0) * P

                    if b == 0 and s == 0 and grp == 0:
                        xt = xt_first
                    else:
                        xt = xt_dma(b, n0, GW)

                    # q^T: stationary = W_q pair block, moving = x^T
                    for p in range(NPAIRS):
                        qtp = qtps.tile([P, GW], F32, tag="qtp")
                        for k in range(NCH):
                            nc.tensor.matmul(
                                qtp[:],
                                lhsT=WQ[k][:, p * P:(p + 1) * P],
                                rhs=xt[:, k, :],
                                start=(k == 0), stop=(k == NCH - 1),
                            )
                        nc.scalar.copy(out=qts[p][:, gt0 * P:gt0 * P + GW],
                                       in_=qtp[:])
                        emit_ph2()

                    for tl in range(gw):
                        t = gt0 + tl
                        ti = soff + t  # global tile index (mask/scale column)

                        kv = kvps.tile([P, 2, INNER], F32, tag="kv")
                        mu_ps = mups.tile([P, 2, HEADS], F32, tag="mu_ps")
                        for k in range(NCH):
                            lx = xt[:, k, tl * P:(tl + 1) * P]
                            nc.tensor.matmul(
                                kv[:, 0, :], lhsT=lx,
                                rhs=WQ[k][:, INNER:2 * INNER],
                                start=(k == 0), stop=(k == NCH - 1))
                            nc.tensor.matmul(
                                kv[:, 1, :], lhsT=lx,
                                rhs=WQ[k][:, 2 * INNER:R],
                                start=(k == 0), stop=(k == NCH - 1))
                            # per-head means, same stationary: mu = x @ Wsum/64
                            nc.tensor.matmul(
                                mu_ps[:].rearrange("p t h -> p (t h)"),
                                lhsT=lx, rhs=WS[:, k, :],
                                start=(k == 0), stop=(k == NCH - 1))

                        kv4 = kv[:].rearrange("p t (h d) -> p t h d", h=HEADS)

                        # LN variance: squares on Act, per-head sums via one
                        # multi-group reduce; mean comes from the matmul above
                        sq = sqpool.tile([P, 2, HEADS, DH + 4], F32, tag="sq")
                        nc.scalar.activation(sq[:, :, :, 0:DH], kv4, Square)
                        mus = stat.tile([P, 2, HEADS], F32, tag="mus")
                        nc.vector.tensor_copy(mus[:], mu_ps[:])
                        msq = stat.tile([P, 2, HEADS], F32, tag="msq")
                        nc.vector.tensor_reduce(out=msq[:],
                                                in_=sq[:, :, :, 0:DH],
                                                axis=X, op=add)
                        D2 = stat.tile([P, 2, HEADS], F32, tag="D2")
                        nc.vector.tensor_tensor(D2[:], mus[:], mus[:], op=mult)
                        var = stat.tile([P, 2, HEADS], F32, tag="var")
                        nc.vector.scalar_tensor_tensor(
                            var[:], msq[:], 1.0 / DH, D2[:], op0=mult, op1=sub)
                        stdt = stat.tile([P, 2, HEADS], F32, tag="stdt")
                        nc.scalar.activation(stdt[:], var[:], Sqrt,
                                             bias=EPSC[:, 0:1])
                        rstd = stat.tile([P, 2, HEADS], F32, tag="rstd")
                        nc.vector.reciprocal(rstd[:], stdt[:])
                        # c = -mu * rstd. No pad mask needed: pad rows have
                        # x = 0, so k = v = mu = 0 and klvl = 0 follows.
                        cv = stat.tile([P, 2, HEADS], BF16, tag="cv")
                        nc.vector.scalar_tensor_tensor(
                            cv[:], mus[:], -1.0, rstd[:], op0=mult, op1=mult)

                        # apply: mult on DVE (PSUM read), add on GpSimd (SBUF)
                        # klvl in bf16: feeds the k^T v matmuls at 1 cycle/row
                        klvl = klvlp.tile([P, 2, HEADS, DH], BF16, tag="klvl")
                        nc.vector.tensor_tensor(
                            klvl[:], kv4,
                            rstd[:, :, :, None].broadcast_to([P, 2, HEADS, DH]),
                            op=mult)
                        nc.gpsimd.tensor_tensor(
                            klvl[:], klvl[:],
                            cv[:, :, :, None].broadcast_to([P, 2, HEADS, DH]),
                            op=add)
                        if ln_general:
                            bmk = stat.tile([P, 2, DH], F32, tag="bmk")
                            nc.vector.tensor_scalar(bmk[:, 0], LNP[:, 1],
                                                    KM[:, ti:ti + 1], None,
                                                    op0=mult)
                            nc.vector.tensor_scalar(bmk[:, 1], LNP[:, 3],
                                                    KM[:, ti:ti + 1], None,
                                                    op0=mult)
                            for half, wi in ((0, 0), (1, 2)):
                                nc.vector.tensor_tensor(
                                    klvl[:, half], klvl[:, half],
                                    LNP[:, wi, None, :].broadcast_to(
                                        [P, HEADS, DH]), op=mult)
                                nc.vector.tensor_tensor(
                                    klvl[:, half], klvl[:, half],
                                    bmk[:, half, None, :].broadcast_to(
                                        [P, HEADS, DH]), op=add)

                        # full-cross pair k^T v (transposed: lhsT = v side),
                        # deferred KTV_LAG tiles to hide the LN chain latency
                        def make_ktv(klvl=klvl, t=t, ktv=ktv, L=L):
                            def go():
                                klf = klvl[:, 0].rearrange("p h d -> p (h d)")
                                vlf = klvl[:, 1].rearrange("p h d -> p (h d)")
                                for p in range(NPAIRS):
                                    nc.tensor.matmul(
                                        ktv[:, p, :],
                                        lhsT=vlf[:, p * P:(p + 1) * P],
                                        rhs=klf[:, p * P:(p + 1) * P],
                                        start=(t == 0 and p == 0),
                                        stop=(t == L - 1 and p == NPAIRS - 1))
                            return go

                        ktv_queue.append((make_ktv(), True))
                        ktv_backlog[0] += 1
                        if ktv_backlog[0] > KTV_LAG:
                            emit_ktv()
                        emit_ph2()

                def make_finalize(b=b, soff=soff, L=L, ktv=ktv, qts=qts):
                    def go():
                        # block-diag (ktv_h)^T for Mf and phase 2
                        bd = bdsb.tile([P, NPAIRS, P], F32R, name="bd",
                                       tag="bd")
                        nc.gpsimd.memset(bd[:].bitcast(mybir.dt.uint32), 0)
                        for p in range(NPAIRS):
                            nc.vector.tensor_copy(bd[0:DH, p, 0:DH],
                                                  ktv[0:DH, p, 0:DH])
                            nc.vector.tensor_copy(bd[DH:P, p, DH:P],
                                                  ktv[DH:P, p, DH:P])
                        # Mf = blockdiag(ktv) @ w_out.T  [INNER-pair x DIM]
                        # (psum space borrowed from the kv pool)
                        Mf = mfsb.tile([P, NPAIRS, DIM], F32R, name="Mf",
                                       tag="Mf")
                        for half in range(2):
                            mfp = kvps.tile([P, 2, INNER], F32, name="mfp",
                                            tag="kv")
                            for i in range(2):
                                p = 2 * half + i
                                nc.tensor.matmul(
                                    mfp[:, i, :], lhsT=bd[:, p, :],
                                    rhs=WO[:, p, :], start=True, stop=True)
                            for i in range(2):
                                p = 2 * half + i
                                nc.scalar.copy(out=Mf[:, p, :],
                                               in_=mfp[:, i, :])
                        for t in range(L):
                            pending_ph2.append(make_ph2(b, soff, t, qts, Mf))
                    return go

                ktv_queue.append((make_finalize(), False))

        while ktv_queue:
            item, is_real = ktv_queue.pop(0)
            item()
        # final drain: nothing left to interleave, so spread the last slot's
        # phase 2 across the now-idle kv psum banks as well
        emit_ph2(len(pending_ph2), pools=(None, kvps))

    nc.compile()
    return nc


# ---------------------------------------------------------------------------
# entry point
# ---------------------------------------------------------------------------

def _run(x, w_qkv, ln1_w, ln1_b, ln2_w, ln2_b, w_out, b_out, batch,
         num_graphs, n_cores=N_CORES, trace=False):
    x = np.ascontiguousarray(np.asarray(x, np.float32))
    counts, starts, core_graphs, Ls = _plan(batch, num_graphs, n_cores)
    per_core, slot_off = _pack_inputs(x, counts, starts, core_graphs, Ls, n_cores)
    T = sum(Ls)

    ln1_w = np.asarray(ln1_w, np.float32)
    ln1_b = np.asarray(ln1_b, np.float32)
    ln2_w = np.asarray(ln2_w, np.float32)
    ln2_b = np.asarray(ln2_b, np.float32)
    ln_general = not (
        np.all(ln1_w == 1.0) and np.all(ln1_b == 0.0)
        and np.all(ln2_w == 1.0) and np.all(ln2_b == 0.0)
    )

    bout_np = np.asarray(b_out, np.float32)
    bo_zero = bool(np.all(bout_np == 0.0))
    key = (T, tuple(Ls), n_cores, ln_general, bo_zero)
    nc = _PROGRAM_CACHE.get(key)
    if nc is None:
        nc = _build_program(T, tuple(Ls), n_cores, ln_general, bo_zero)
        _PROGRAM_CACHE[key] = nc

    wqkvT = np.ascontiguousarray(np.asarray(w_qkv, np.float32).T)
    woutT = np.ascontiguousarray(np.asarray(w_out, np.float32).T)
    bout = np.ascontiguousarray(np.asarray(b_out, np.float32))
    lnp = np.stack([ln1_w, ln1_b, ln2_w, ln2_b])
    # per-head column means of W_k | W_v: mean(k)[n,h] = x[n] @ wsum[:, h]
    wsum = np.ascontiguousarray(
        wqkvT[:, INNER:].reshape(DIM, 2, HEADS, DH).mean(-1).reshape(DIM, -1))

    in_maps = []
    for c in range(n_cores):
        xTp, qscv, kvmv = per_core[c]
        m = {"xT": xTp, "wqkvT": wqkvT, "wsum": wsum, "woutT": woutT,
             "bout": bout, "qsc": qscv, "kvm": kvmv}
        if ln_general:
            m["lnp"] = lnp
        in_maps.append(m)

    res = run_bass_kernel_spmd(nc, in_maps, list(range(n_cores)), trace=trace)

    N = x.shape[1]
    y = np.empty((B, N, DIM), np.float32)
    for c in range(n_cores):
        oc = res.results[c]["out"]
        for s, g in enumerate(core_graphs[c]):
            if g < 0 or counts[g] == 0:
                continue
            n0, ng = int(starts[g]), int(counts[g])
            off = int(slot_off[s]) * P
            y[:, n0:n0 + ng, :] = oc[:, off:off + ng, :]
    return y, res


def kernel(**inputs):
    trace = bool(os.environ.get("GALERKIN_TRACE"))
    y, _ = _run(
        inputs["x"], inputs["w_qkv"], inputs["ln1_w"], inputs["ln1_b"],
        inputs["ln2_w"], inputs["ln2_b"], inputs["w_out"], inputs["b_out"],
        inputs["batch"], inputs["num_graphs"], trace=trace,
    )
    return y


# revision 40
# speedup vs baseline: 3.4359x; 1.1449x over previous
"""Galerkin attention (ragged graph segments) on 8 Trainium2 NeuronCores.

Math (per reference):
  qkv = x @ w_qkv.T ; split q,k,v -> [B, H, N, DH]
  k, v  <- LayerNorm over DH (eps=1e-6, affine)
  per graph g (sorted contiguous segments of N): ktv[g] = k_g^T v_g
  out_n = (q_n / size(g(n))) @ ktv[g(n)]
  y = out @ w_out.T + b_out

Sharding: 32 graphs are assigned to 8 cores x S slots by sorted rank (only
the per-slot maxima matter under SPMD); every core runs the identical
instruction stream over T = sum(L_s) 128-row tiles per batch entry. Ragged
graph ends are zero-padded; zero rows yield mu = k = v = 0 so LN emits
exact zeros without masking, and the per-node output scale (1/graph size,
zero on pads) handles the q side.

Dataflow per 128-node tile (engines balanced, PE kept streaming):
  PE : qkv projection (f32r, 512-free), per-head LN means via a folded
       wsum matmul, full-cross pair k^T v (bf16, 128-free, one PSUM bank
       per slot), per-slot Mf = blockdiag(ktv) @ w_out.T, and phase 2
       out = (q^T)^T @ Mf (the out-projection is pre-folded into Mf).
  Act: squares for LN variance, sqrt(var+eps), psum->sbuf copies, out scale.
  DVE: one multi-group tensor_reduce (sum k^2 per head), tiny fixups, and
       the broadcast LN multiply (per-node-head scale via stride-0 AP).
  Pool(GpSimd): broadcast LN add (SBUF only; GPSIMD cannot touch PSUM).
The k^T v matmuls trail the projection by KTV_LAG tiles (carried across
slot boundaries), and phase 2 of slot s-1 is emitted interleaved into
phase 1 of slot s, so the in-order PE queue always has ready matmuls
while the LN chain drains.
"""

import os
import sys

if "/opt/trn_rl_repo" not in sys.path:
    sys.path.insert(0, "/opt/trn_rl_repo")

import numpy as np

import concourse.bacc as bacc
import concourse.bass as bass
import concourse.mybir as mybir
import concourse.tile as tile
from concourse.bass_utils import run_bass_kernel_spmd

P = 128
B = 2
DIM = 512
HEADS = 8
DH = 64
INNER = HEADS * DH          # 512
R = 3 * INNER               # 1536
NCH = DIM // P              # 4 contraction chunks
NPAIRS = HEADS // 2         # 4 head pairs
EPS = 1e-6
N_CORES = 8
GRP = 4                     # tiles per matmul group (512-node span)
KTV_LAG = 4                 # tiles of software pipelining before k^T v
F32 = mybir.dt.float32
F32R = mybir.dt.float32r
BF16 = mybir.dt.bfloat16

_PROGRAM_CACHE: dict = {}


def _revap(src):
    """View of `src` with the innermost (contiguous) dim reversed.

    Stats are permutation-invariant; the negative stride stops the AP
    optimizer from merging the per-head groups into one flat run, which
    would turn a multi-group tensor_reduce into a single global one.
    """
    inner = list(src.ap[-1])
    assert inner[0] == 1 and inner[1] == DH
    return bass.AP(
        tensor=src.tensor,
        offset=src.offset + (DH - 1),
        ap=[list(d) for d in src.ap[:-1]] + [[-1, DH]],
    )


# ---------------------------------------------------------------------------
# host-side planning
# ---------------------------------------------------------------------------

def _plan(batch, num_graphs, n_cores):
    """Assign graphs to (core, slot) and compute the uniform slot widths."""
    batch = np.asarray(batch).astype(np.int64)
    G = int(num_graphs)
    counts = np.bincount(batch, minlength=G)[:G].astype(np.int64)
    starts = np.concatenate([[0], np.cumsum(counts)[:-1]])
    tiles_g = (counts + P - 1) // P

    # SPMD: every core executes the same T = sum_s max_c tiles, so only the
    # per-slot maxima matter. Sorting by size and filling slot s with ranks
    # [s*n_cores, (s+1)*n_cores) minimizes each slot's max simultaneously.
    S = (G + n_cores - 1) // n_cores
    order = list(np.argsort(-tiles_g, kind="stable")) + [-1] * (S * n_cores - G)
    core_graphs = [[] for _ in range(n_cores)]
    for s in range(S):
        for c in range(n_cores):
            core_graphs[c].append(int(order[s * n_cores + c]))

    Ls = []
    for s in range(S):
        L = max(
            int(tiles_g[core_graphs[c][s]]) if core_graphs[c][s] >= 0 else 0
            for c in range(n_cores)
        )
        Ls.append(max(L, 1))
    return counts, starts, core_graphs, Ls


def _pack_inputs(x, counts, starts, core_graphs, Ls, n_cores):
    T = sum(Ls)
    slot_off = np.concatenate([[0], np.cumsum(Ls)[:-1]])
    xT = np.ascontiguousarray(np.transpose(x, (0, 2, 1)))  # [B, DIM, N]
    per_core = []
    for c in range(n_cores):
        xTp = np.zeros((B, DIM, T * P), np.float32)
        qsc = np.zeros((T * P,), np.float32)
        kvm = np.zeros((T * P,), np.float32)
        for s, g in enumerate(core_graphs[c]):
            if g < 0 or counts[g] == 0:
                continue
            n0, ng = int(starts[g]), int(counts[g])
            off = int(slot_off[s]) * P
            xTp[:, :, off:off + ng] = xT[:, :, n0:n0 + ng]
            qsc[off:off + ng] = 1.0 / ng
            kvm[off:off + ng] = 1.0
        per_core.append((xTp, qsc, kvm))
    return per_core, slot_off


# ---------------------------------------------------------------------------
# device program
# ---------------------------------------------------------------------------

def _build_program(T, Ls, n_cores, ln_general, bo_zero=False):
    from contextlib import ExitStack

    nc = bacc.Bacc("TRN2", target_bir_lowering=False, debug=False,
                   num_devices=n_cores)

    xT = nc.dram_tensor("xT", [B, DIM, T * P], F32R, kind="ExternalInput")
    wq = nc.dram_tensor("wqkvT", [DIM, R], F32R, kind="ExternalInput")
    ws = nc.dram_tensor("wsum", [DIM, 2 * HEADS], F32R, kind="ExternalInput")
    wo = nc.dram_tensor("woutT", [INNER, DIM], F32R, kind="ExternalInput")
    bo = nc.dram_tensor("bout", [DIM], F32, kind="ExternalInput")
    qsc = nc.dram_tensor("qsc", [T * P], F32, kind="ExternalInput")
    kvm = nc.dram_tensor("kvm", [T * P], F32, kind="ExternalInput")
    if ln_general:
        lnp = nc.dram_tensor("lnp", [4, DH], F32, kind="ExternalInput")
    out = nc.dram_tensor("out", [B, T * P, DIM], F32, kind="ExternalOutput")

    slot_off = [0]
    for L in Ls[:-1]:
        slot_off.append(slot_off[-1] + L)

    Sqrt = mybir.ActivationFunctionType.Sqrt
    Square = mybir.ActivationFunctionType.Square
    mult = mybir.AluOpType.mult
    add = mybir.AluOpType.add
    sub = mybir.AluOpType.subtract
    X = mybir.AxisListType.X

    with ExitStack() as ctx:
        tc = ctx.enter_context(tile.TileContext(nc))
        const = ctx.enter_context(tc.tile_pool(name="const", bufs=1))

        xpool = ctx.enter_context(tc.tile_pool(name="xp", bufs=4))
        xTr = xT.ap()

        def xt_dma(b, n0, GW):
            t = xpool.tile([P, NCH, GW], F32R, name="xt", tag="xt")
            nc.sync.dma_start(
                out=t[:],
                in_=xTr[b].rearrange("(k c) n -> c k n", c=P)[:, :, n0:n0 + GW],
            )
            return t

        # first x tile group + W chunk 0 land before the bulk constants so
        # the PE pipeline starts ~20us earlier
        xt_first = xt_dma(0, 0, min(GRP, Ls[0]) * P)
        WQ = [const.tile([P, R], F32R, name=f"WQ{k}", tag=f"WQ{k}")
              for k in range(NCH)]
        wqr = wq.ap().rearrange("(k c) r -> k c r", c=P)
        for k in range(NCH):
            nc.sync.dma_start(out=WQ[k][:], in_=wqr[k])
        WS = const.tile([P, NCH, 2 * HEADS], F32R, tag="WS")
        nc.sync.dma_start(out=WS[:], in_=ws.ap().rearrange("(k c) r -> c k r", c=P))
        WO = const.tile([P, NCH, DIM], F32R, tag="WO")
        nc.sync.dma_start(out=WO[:], in_=wo.ap().rearrange("(k c) d -> c k d", c=P))
        BO = const.tile([P, DIM], F32, tag="BO")
        nc.sync.dma_start(out=BO[:], in_=bo.ap().partition_broadcast(P))
        QS = const.tile([P, T], F32, tag="QS")
        nc.sync.dma_start(out=QS[:], in_=qsc.ap().rearrange("(t p) -> p t", p=P))
        KM = const.tile([P, T], F32, tag="KM")
        nc.sync.dma_start(out=KM[:], in_=kvm.ap().rearrange("(t p) -> p t", p=P))
        EPSC = const.tile([P, 1], F32, tag="EPSC")
        nc.vector.memset(EPSC[:], EPS)
        if ln_general:
            LNP = const.tile([P, 4, DH], F32, tag="LNP")
            nc.sync.dma_start(out=LNP[:], in_=lnp.ap().partition_broadcast(P))

        sqpool = ctx.enter_context(tc.tile_pool(name="sqp", bufs=3))
        klvlp = ctx.enter_context(tc.tile_pool(name="klvlp", bufs=KTV_LAG + 2))
        stat = ctx.enter_context(tc.tile_pool(name="stat", bufs=27))
        qstash = ctx.enter_context(tc.tile_pool(name="qstash", bufs=3 * NPAIRS))
        bdsb = ctx.enter_context(tc.tile_pool(name="bd", bufs=2))
        mfsb = ctx.enter_context(tc.tile_pool(name="mf", bufs=2))
        outsb = ctx.enter_context(tc.tile_pool(name="outsb", bufs=4))

        kvps = ctx.enter_context(tc.tile_pool(name="kvps", bufs=2, space="PSUM"))
        qtps = ctx.enter_context(tc.tile_pool(name="qtps", bufs=1, space="PSUM"))
        ktps = ctx.enter_context(tc.tile_pool(name="ktps", bufs=1, space="PSUM"))
        mips = ctx.enter_context(tc.tile_pool(name="mips", bufs=1, space="PSUM"))
        mups = ctx.enter_context(tc.tile_pool(name="mups", bufs=1, space="PSUM"))

        # phase-2 emitters for the previous slot, interleaved into the next
        # slot's phase 1 to keep the PE queue stocked with ready matmuls
        pending_ph2 = []

        def emit_ph2(k=1, pools=(None,)):
            for i in range(k):
                if pending_ph2:
                    pending_ph2.pop(0)(pools[i % len(pools)])

        def make_ph2(b, soff, t, qts, Mf):
            ti = soff + t

            def go(ps_pool=None):
                if ps_pool is None:
                    opsv = mips.tile([P, DIM], F32, name="ops", tag="mi")[:]
                else:
                    opsv = ps_pool.tile([P, 2, INNER], F32, name="ops",
                                        tag="kv")[:, 0, :]
                for p in range(NPAIRS):
                    nc.tensor.matmul(
                        opsv,
                        lhsT=qts[p][:, t * P:(t + 1) * P],
                        rhs=Mf[:, p, :],
                        start=(p == 0), stop=(p == NPAIRS - 1),
                    )
                ot = outsb.tile([P, DIM], F32, tag="ot")
                if bo_zero:
                    nc.scalar.mul(ot[:], opsv, QS[:, ti:ti + 1])
                else:
                    nc.vector.scalar_tensor_tensor(
                        ot[:], opsv, QS[:, ti:ti + 1], BO[:],
                        op0=mult, op1=add)
                nc.sync.dma_start(
                    out=out.ap()[b, ti * P:(ti + 1) * P, :], in_=ot[:])

            return go

        # global deferred k^T v queue: emitters stay KTV_LAG tiles behind and
        # carry over slot boundaries; a finalizer item per slot emits the
        # bd/Mf reduction and queues that slot's phase 2
        ktv_queue = []
        ktv_backlog = [0]

        def emit_ktv():
            while ktv_queue:
                item, is_real = ktv_queue.pop(0)
                item()
                if is_real:
                    ktv_backlog[0] -= 1
                    return

        for b in range(B):
            for s, L in enumerate(Ls):
                soff = slot_off[s]
                ktv = ktps.tile([P, NPAIRS, P], F32, tag="ktv")
                qts = [qstash.tile([P, L * P], F32R, name=f"qts{i}", tag="qstash")
                       for i in range(NPAIRS)]
                ngroups = (L + GRP - 1) // GRP

                for grp in range(ngroups):
                    gt0 = grp * GRP
                    gw = min(GRP, L - gt0)
                    GW = gw * P
                    n0 = (soff + gt0) * P

                    if b == 0 and s == 0 and grp == 0:
                        xt = xt_first
                    else:
                        xt = xt_dma(b, n0, GW)

                    # q^T: stationary = W_q pair block, moving = x^T
                    for p in range(NPAIRS):
                        qtp = qtps.tile([P, GW], F32, tag="qtp")
                        for k in range(NCH):
                            nc.tensor.matmul(
                                qtp[:],
                                lhsT=WQ[k][:, p * P:(p + 1) * P],
                                rhs=xt[:, k, :],
                                start=(k == 0), stop=(k == NCH - 1),
                            )
                        nc.scalar.copy(out=qts[p][:, gt0 * P:gt0 * P + GW],
                                       in_=qtp[:])

                    for tl in range(gw):
                        t = gt0 + tl
                        ti = soff + t  # global tile index (mask/scale column)

                        kv = kvps.tile([P, 2, INNER], F32, tag="kv")
                        mu_ps = mups.tile([P, 2, HEADS], F32, tag="mu_ps")
                        for k in range(NCH):
                            lx = xt[:, k, tl * P:(tl + 1) * P]
                            nc.tensor.matmul(
                                kv[:, 0, :], lhsT=lx,
                                rhs=WQ[k][:, INNER:2 * INNER],
                                start=(k == 0), stop=(k == NCH - 1))
                            nc.tensor.matmul(
                                kv[:, 1, :], lhsT=lx,
                                rhs=WQ[k][:, 2 * INNER:R],
                                start=(k == 0), stop=(k == NCH - 1))
                            # per-head means, same stationary: mu = x @ Wsum/64
                            nc.tensor.matmul(
                                mu_ps[:].rearrange("p t h -> p (t h)"),
                                lhsT=lx, rhs=WS[:, k, :],
                                start=(k == 0), stop=(k == NCH - 1))

                        kv4 = kv[:].rearrange("p t (h d) -> p t h d", h=HEADS)

                        # LN variance: squares on Act, per-head sums via one
                        # multi-group reduce; mean comes from the matmul above
                        sq = sqpool.tile([P, 2, HEADS, DH + 4], F32, tag="sq")
                        nc.scalar.activation(sq[:, :, :, 0:DH], kv4, Square)
                        mus = stat.tile([P, 2, HEADS], F32, tag="mus")
                        nc.vector.tensor_copy(mus[:], mu_ps[:])
                        msq = stat.tile([P, 2, HEADS], F32, tag="msq")
                        nc.vector.tensor_reduce(out=msq[:],
                                                in_=sq[:, :, :, 0:DH],
                                                axis=X, op=add)
                        D2 = stat.tile([P, 2, HEADS], F32, tag="D2")
                        nc.vector.tensor_tensor(D2[:], mus[:], mus[:], op=mult)
                        var = stat.tile([P, 2, HEADS], F32, tag="var")
                        nc.vector.scalar_tensor_tensor(
                            var[:], msq[:], 1.0 / DH, D2[:], op0=mult, op1=sub)
                        stdt = stat.tile([P, 2, HEADS], F32, tag="stdt")
                        nc.scalar.activation(stdt[:], var[:], Sqrt,
                                             bias=EPSC[:, 0:1])
                        rstd = stat.tile([P, 2, HEADS], F32, tag="rstd")
                        nc.vector.reciprocal(rstd[:], stdt[:])
                        # c = -mu * rstd. No pad mask needed: pad rows have
                        # x = 0, so k = v = mu = 0 and klvl = 0 follows.
                        cv = stat.tile([P, 2, HEADS], BF16, tag="cv")
                        nc.vector.scalar_tensor_tensor(
                            cv[:], mus[:], -1.0, rstd[:], op0=mult, op1=mult)

                        # apply: mult on DVE (PSUM read), add on GpSimd (SBUF)
                        # klvl in bf16: feeds the k^T v matmuls at 1 cycle/row
                        klvl = klvlp.tile([P, 2, HEADS, DH], BF16, tag="klvl")
                        nc.vector.tensor_tensor(
                            klvl[:], kv4,
                            rstd[:, :, :, None].broadcast_to([P, 2, HEADS, DH]),
                            op=mult)
                        nc.gpsimd.tensor_tensor(
                            klvl[:], klvl[:],
                            cv[:, :, :, None].broadcast_to([P, 2, HEADS, DH]),
                            op=add)
                        if ln_general:
                            bmk = stat.tile([P, 2, DH], F32, tag="bmk")
                            nc.vector.tensor_scalar(bmk[:, 0], LNP[:, 1],
                                                    KM[:, ti:ti + 1], None,
                                                    op0=mult)
                            nc.vector.tensor_scalar(bmk[:, 1], LNP[:, 3],
                                                    KM[:, ti:ti + 1], None,
                                                    op0=mult)
                            for half, wi in ((0, 0), (1, 2)):
                                nc.vector.tensor_tensor(
                                    klvl[:, half], klvl[:, half],
                                    LNP[:, wi, None, :].broadcast_to(
                                        [P, HEADS, DH]), op=mult)
                                nc.vector.tensor_tensor(
                                    klvl[:, half], klvl[:, half],
                                    bmk[:, half, None, :].broadcast_to(
                                        [P, HEADS, DH]), op=add)

                        # full-cross pair k^T v (transposed: lhsT = v side),
                        # deferred KTV_LAG tiles to hide the LN chain latency
                        def make_ktv(klvl=klvl, t=t, ktv=ktv, L=L):
                            def go():
                                klf = klvl[:, 0].rearrange("p h d -> p (h d)")
                                vlf = klvl[:, 1].rearrange("p h d -> p (h d)")
                                for p in range(NPAIRS):
                                    nc.tensor.matmul(
                                        ktv[:, p, :],
                                        lhsT=vlf[:, p * P:(p + 1) * P],
                                        rhs=klf[:, p * P:(p + 1) * P],
                                        start=(t == 0 and p == 0),
                                        stop=(t == L - 1 and p == NPAIRS - 1))
                            return go

                        ktv_queue.append((make_ktv(), True))
                        ktv_backlog[0] += 1
                        if ktv_backlog[0] > KTV_LAG:
                            emit_ktv()
                        emit_ph2()

                def make_finalize(b=b, soff=soff, L=L, ktv=ktv, qts=qts):
                    def go():
                        # block-diag (ktv_h)^T for Mf and phase 2
                        bd = bdsb.tile([P, NPAIRS, P], F32R, name="bd",
                                       tag="bd")
                        nc.gpsimd.memset(bd[:].bitcast(mybir.dt.uint32), 0)
                        for p in range(NPAIRS):
                            nc.vector.tensor_copy(bd[0:DH, p, 0:DH],
                                                  ktv[0:DH, p, 0:DH])
                            nc.vector.tensor_copy(bd[DH:P, p, DH:P],
                                                  ktv[DH:P, p, DH:P])
                        # Mf = blockdiag(ktv) @ w_out.T  [INNER-pair x DIM]
                        # (psum space borrowed from the kv pool)
                        Mf = mfsb.tile([P, NPAIRS, DIM], F32R, name="Mf",
                                       tag="Mf")
                        for half in range(2):
                            mfp = kvps.tile([P, 2, INNER], F32, name="mfp",
                                            tag="kv")
                            for i in range(2):
                                p = 2 * half + i
                                nc.tensor.matmul(
                                    mfp[:, i, :], lhsT=bd[:, p, :],
                                    rhs=WO[:, p, :], start=True, stop=True)
                            for i in range(2):
                                p = 2 * half + i
                                nc.scalar.copy(out=Mf[:, p, :],
                                               in_=mfp[:, i, :])
                        for t in range(L):
                            pending_ph2.append(make_ph2(b, soff, t, qts, Mf))
                    return go

                ktv_queue.append((make_finalize(), False))

        while ktv_queue:
            item, is_real = ktv_queue.pop(0)
            item()
        # final drain: nothing left to interleave, so spread the last slot's
        # phase 2 across the now-idle kv psum banks as well
        emit_ph2(len(pending_ph2), pools=(None, kvps))

    nc.compile()
    return nc


# ---------------------------------------------------------------------------
# entry point
# ---------------------------------------------------------------------------

def _run(x, w_qkv, ln1_w, ln1_b, ln2_w, ln2_b, w_out, b_out, batch,
         num_graphs, n_cores=N_CORES, trace=False):
    x = np.ascontiguousarray(np.asarray(x, np.float32))
    counts, starts, core_graphs, Ls = _plan(batch, num_graphs, n_cores)
    per_core, slot_off = _pack_inputs(x, counts, starts, core_graphs, Ls, n_cores)
    T = sum(Ls)

    ln1_w = np.asarray(ln1_w, np.float32)
    ln1_b = np.asarray(ln1_b, np.float32)
    ln2_w = np.asarray(ln2_w, np.float32)
    ln2_b = np.asarray(ln2_b, np.float32)
    ln_general = not (
        np.all(ln1_w == 1.0) and np.all(ln1_b == 0.0)
        and np.all(ln2_w == 1.0) and np.all(ln2_b == 0.0)
    )

    bout_np = np.asarray(b_out, np.float32)
    bo_zero = bool(np.all(bout_np == 0.0))
    key = (T, tuple(Ls), n_cores, ln_general, bo_zero)
    nc = _PROGRAM_CACHE.get(key)
    if nc is None:
        nc = _build_program(T, tuple(Ls), n_cores, ln_general, bo_zero)
        _PROGRAM_CACHE[key] = nc

    wqkvT = np.ascontiguousarray(np.asarray(w_qkv, np.float32).T)
    woutT = np.ascontiguousarray(np.asarray(w_out, np.float32).T)
    bout = np.ascontiguousarray(np.asarray(b_out, np.float32))
    lnp = np.stack([ln1_w, ln1_b, ln2_w, ln2_b])
    # per-head column means of W_k | W_v: mean(k)[n,h] = x[n] @ wsum[:, h]
    wsum = np.ascontiguousarray(
        wqkvT[:, INNER:].reshape(DIM, 2, HEADS, DH).mean(-1).reshape(DIM, -1))

    in_maps = []
    for c in range(n_cores):
        xTp, qscv, kvmv = per_core[c]
        m = {"xT": xTp, "wqkvT": wqkvT, "wsum": wsum, "woutT": woutT,
             "bout": bout, "qsc": qscv, "kvm": kvmv}
        if ln_general:
            m["lnp"] = lnp
        in_maps.append(m)

    res = run_bass_kernel_spmd(nc, in_maps, list(range(n_cores)), trace=trace)

    N = x.shape[1]
    y = np.empty((B, N, DIM), np.float32)
    for c in range(n_cores):
        oc = res.results[c]["out"]
        for s, g in enumerate(core_graphs[c]):
            if g < 0 or counts[g] == 0:
                continue
            n0, ng = int(starts[g]), int(counts[g])
            off = int(slot_off[s]) * P
            y[:, n0:n0 + ng, :] = oc[:, off:off + ng, :]
    return y, res


def kernel(**inputs):
    trace = bool(os.environ.get("GALERKIN_TRACE"))
    y, _ = _run(
        inputs["x"], inputs["w_qkv"], inputs["ln1_w"], inputs["ln1_b"],
        inputs["ln2_w"], inputs["ln2_b"], inputs["w_out"], inputs["b_out"],
        inputs["batch"], inputs["num_graphs"], trace=trace,
    )
    return y
